# Initial kernel scaffold
#
"""Self-contained Trainium2 kernel for nn_ActionQuantizer (VQ-VAE action quantizer).

Strategy: data-parallel over the N=65536 batch across 8 NeuronCores.
- Encoder/decoder MLPs run feature-major with fp16 hi/lo x3 matmuls (~fp32 precision).
- VQ distances computed twice: row-major (argmax/onehot/q) and column-sharded
  (64 codebook columns per core over the full batch, via an AllGather of zn)
  for the contrastive-loss order statistics.
- Contrastive sort stats via threshold selection: segment-top8 candidates ->
  bisect for the top-n_pos threshold; strided-subsample bisect for the median;
  exact correction terms make the result insensitive to small threshold error.
- Scalar losses all-reduced on device; host only reassembles shards.
"""
import sys

sys.path.insert(0, "/opt/trn_rl_repo")
import numpy as np
import ml_dtypes  # noqa: F401

N_TOT = 65536
NCORE = 8
ACTD = 12
COND = 512
H1 = 512
H2 = 256
EMB = 16
K = 512
CC = 0.25


def f16(x):
    return np.asarray(x, np.float32).astype(np.float16)


def f16lo(x):
    x = np.asarray(x, np.float32)
    return (x - x.astype(np.float16).astype(np.float32)).astype(np.float16)


def build(ns, nblk):
    """Build the SPMD Bass program for ns rows per core, streaming block nblk."""
    import concourse.bass as bass
    import concourse.tile as tile
    from concourse import bacc, mybir
    from concourse import bass_isa

    F32 = mybir.dt.float32
    F16 = mybir.dt.float16
    I32 = mybir.dt.int32
    Alu = mybir.AluOpType
    Act = mybir.ActivationFunctionType

    NT = ns // 128            # row tiles per core
    NBT = ns // nblk          # blocks per core
    TPB = nblk // 128         # row tiles per block
    LH = 4 * ns               # per-partition half-column length of Dt (= N/2)
    SEG = min(1024, LH)
    NSEG = LH // SEG
    CW = 2 * NSEG * 8         # candidate width after halves merge
    NPOS = (NCORE * ns) // K  # top-n_pos for dis_pos
    SST = 16                  # median subsample stride
    SUBW = LH // SST
    NCHUNK = 8                # chunks for full-Dt passes
    CH = LH // NCHUNK
    NFULL = NCORE * ns

    nc = bacc.Bacc("TRN2", target_bir_lowering=False, debug=False,
                   num_devices=NCORE)

    def din(name, shape, dt=F32):
        return nc.dram_tensor(name, shape, dt, kind="ExternalInput").ap()

    def dout(name, shape, dt=F32):
        return nc.dram_tensor(name, shape, dt, kind="ExternalOutput").ap()

    # ---- inputs ----
    a32_d = din("a32", [ns, ACTD])
    ah_d = din("ah", [ns, ACTD], F16)
    al_d = din("al", [ns, ACTD], F16)
    ch_d = din("ch", [ns, COND], F16)
    cl_d = din("cl", [ns, COND], F16)
    w1c_h_d = din("w1c_h", [128, 4 * H1], F16)
    w1c_l_d = din("w1c_l", [128, 4 * H1], F16)
    w1a_h_d = din("w1a_h", [ACTD, H1], F16)
    w1a_l_d = din("w1a_l", [ACTD, H1], F16)
    b1_d = din("b1", [128, 4])
    nb1_d = din("nb1", [128, 4])
    w2_h_d = din("w2_h", [128, 8 * 128], F16)
    w2_l_d = din("w2_l", [128, 8 * 128], F16)
    b2_d = din("b2", [128, 2])
    nb2_d = din("nb2", [128, 2])
    w3_h_d = din("w3_h", [128, 2 * EMB], F16)
    w3_l_d = din("w3_l", [128, 2 * EMB], F16)
    b3_h_d = din("b3_h", [1, EMB], F16)
    b3_l_d = din("b3_l", [1, EMB], F16)
    ent_h_d = din("ent_h", [EMB, K], F16)
    ent_l_d = din("ent_l", [EMB, K], F16)
    ensl_h_d = din("ensl_h", [EMB, 64], F16)   # per-core column slice of EnT
    ensl_l_d = din("ensl_l", [EMB, 64], F16)
    e_h_d = din("e_h", [128, 4 * EMB], F16)
    e_l_d = din("e_l", [128, 4 * EMB], F16)
    wd1c_h_d = din("wd1c_h", [128, 8 * 128], F16)
    wd1c_l_d = din("wd1c_l", [128, 8 * 128], F16)
    wd1q_d = din("wd1q", [EMB, H2], F16)
    bd1_d = din("bd1", [128, 2])
    nbd1_d = din("nbd1", [128, 2])
    wd2_h_d = din("wd2_h", [128, 8 * 128], F16)
    wd2_l_d = din("wd2_l", [128, 8 * 128], F16)
    bd2_d = din("bd2", [128, 4])
    nbd2_d = din("nbd2", [128, 4])
    wd3_h_d = din("wd3_h", [128, 4 * ACTD], F16)
    wd3_l_d = din("wd3_l", [128, 4 * ACTD], F16)
    bd3_h_d = din("bd3_h", [1, ACTD], F16)
    bd3_l_d = din("bd3_l", [1, ACTD], F16)
    iota_d = din("iota", [128, K])

    # ---- outputs ----
    recon_d = dout("recon", [ns, ACTD])
    qst_d = dout("qst", [ns, EMB])
    idx_d = dout("idx", [ns], I32)
    sc_d = dout("scal", [1, 8])

    # ---- internal DRAM ----
    znh_d = nc.dram_tensor("znh_b", [ns, EMB], F16).ap()
    znl_d = nc.dram_tensor("znl_b", [ns, EMB], F16).ap()
    zgh_d = nc.dram_tensor("zgh_b", [NFULL, EMB], F16).ap()
    zgl_d = nc.dram_tensor("zgl_b", [NFULL, EMB], F16).ap()
    oh_d = nc.dram_tensor("oh_b", [ns, K], F16).ap()
    q16_d = nc.dram_tensor("q16_b", [ns, EMB], F16).ap()
    sci_d = nc.dram_tensor("sci_b", [1, 520]).ap()
    sco_d = nc.dram_tensor("sco_b", [1, 520]).ap()

    groups = [list(range(NCORE))]

    with tile.TileContext(nc) as tc:
        res = tc.tile_pool(name="res", bufs=1).__enter__()
        xin = tc.tile_pool(name="xin", bufs=2).__enter__()
        hwork = tc.tile_pool(name="hwork", bufs=2).__enter__()
        scr = tc.tile_pool(name="scr", bufs=4).__enter__()
        scrs = tc.tile_pool(name="scrs", bufs=3).__enter__()
        pmm = tc.tile_pool(name="pmm", bufs=2, space="PSUM").__enter__()
        psm = tc.tile_pool(name="psm", bufs=3, space="PSUM").__enter__()
        pk1 = tc.tile_pool(name="pk1", bufs=1, space="PSUM").__enter__()

        def rtile(shape, dt, tag):
            return res.tile(shape, dt, tag=tag)

        # ---- load constants / weights ----
        def load(dram, shape, dt, tag):
            t = rtile(shape, dt, tag)
            nc.sync.dma_start(out=t[:], in_=dram[:])
            return t

        w1c_h = load(w1c_h_d, [128, 4 * H1], F16, "w1ch")
        w1c_l = load(w1c_l_d, [128, 4 * H1], F16, "w1cl")
        w1a_h = load(w1a_h_d, [ACTD, H1], F16, "w1ah")
        w1a_l = load(w1a_l_d, [ACTD, H1], F16, "w1al")
        b1 = load(b1_d, [128, 4], F32, "b1")
        nb1 = load(nb1_d, [128, 4], F32, "nb1")
        w2_h = load(w2_h_d, [128, 8 * 128], F16, "w2h")
        w2_l = load(w2_l_d, [128, 8 * 128], F16, "w2l")
        b2 = load(b2_d, [128, 2], F32, "b2")
        nb2 = load(nb2_d, [128, 2], F32, "nb2")
        w3_h = load(w3_h_d, [128, 2 * EMB], F16, "w3h")
        w3_l = load(w3_l_d, [128, 2 * EMB], F16, "w3l")
        b3_h = load(b3_h_d, [1, EMB], F16, "b3h")
        b3_l = load(b3_l_d, [1, EMB], F16, "b3l")
        ent_h = load(ent_h_d, [EMB, K], F16, "enth")
        ent_l = load(ent_l_d, [EMB, K], F16, "entl")
        ensl_h = load(ensl_h_d, [EMB, 64], F16, "enslh")
        ensl_l = load(ensl_l_d, [EMB, 64], F16, "ensll")
        e_h = load(e_h_d, [128, 4 * EMB], F16, "eh")
        e_l = load(e_l_d, [128, 4 * EMB], F16, "el")
        wd1c_h = load(wd1c_h_d, [128, 8 * 128], F16, "wd1ch")
        wd1c_l = load(wd1c_l_d, [128, 8 * 128], F16, "wd1cl")
        wd1q = load(wd1q_d, [EMB, H2], F16, "wd1q")
        bd1 = load(bd1_d, [128, 2], F32, "bd1")
        nbd1 = load(nbd1_d, [128, 2], F32, "nbd1")
        wd2_h = load(wd2_h_d, [128, 8 * 128], F16, "wd2h")
        wd2_l = load(wd2_l_d, [128, 8 * 128], F16, "wd2l")
        bd2 = load(bd2_d, [128, 4], F32, "bd2")
        nbd2 = load(nbd2_d, [128, 4], F32, "nbd2")
        wd3_h = load(wd3_h_d, [128, 4 * ACTD], F16, "wd3h")
        wd3_l = load(wd3_l_d, [128, 4 * ACTD], F16, "wd3l")
        bd3_h = load(bd3_h_d, [1, ACTD], F16, "bd3h")
        bd3_l = load(bd3_l_d, [1, ACTD], F16, "bd3l")
        iota = load(iota_d, [128, K], F32, "iota")
        act_sb = load(a32_d.rearrange("(t p) e -> p (t e)", p=128),
                      [128, NT * ACTD], F32, "actsb")

        ones_r = rtile([1, 128], F16, "ones_r")
        nc.vector.memset(ones_r[:], 1.0)
        ones_c = rtile([128, 1], F16, "ones_c")
        nc.vector.memset(ones_c[:], 1.0)
        ones_c32 = rtile([64, 1], F32, "ones_c32")
        nc.vector.memset(ones_c32[:], 1.0)

        # actions transposed (tiny, full-ns resident)
        aT_h = rtile([ACTD, ns], F16, "aTh")
        aT_l = rtile([ACTD, ns], F16, "aTl")
        nc.sync.dma_start_transpose(aT_h[:], ah_d[:])
        nc.sync.dma_start_transpose(aT_l[:], al_d[:])

        # persistent state
        zsb = rtile([128, NT * EMB], F32, "zsb")
        q_sb = rtile([128, NT * EMB], F32, "qsb")
        q16_sb = rtile([128, NT * EMB], F16, "q16sb")
        idx_sb = rtile([128, NT], F32, "idxsb")
        recon_sb = rtile([128, NT * ACTD], F32, "reconsb")
        znh_sb = rtile([128, NT * EMB], F16, "znhsb")
        znl_sb = rtile([128, NT * EMB], F16, "znlsb")
        Dt = rtile([128, LH], F16, "Dt")
        oh_full = rtile([128, NT * K], F16, "ohfull")

        # ---------- ELU' helper: out = elu(pre+b)+1, split hi/lo f16 ----------
        def elu_split(ps, fwid, bap, nbap, hi_out, lo_out):
            e = scr.tile([128, fwid], F32, tag="elu_e")
            nc.scalar.activation(e[:], ps, Act.Exp, bias=bap, scale=1.0)
            em = scr.tile([128, fwid], F32, tag="elu_m")
            nc.gpsimd.tensor_scalar(out=em[:], in0=e[:], scalar1=1.0,
                                    scalar2=bap, op0=Alu.min, op1=Alu.add)
            hf = scr.tile([128, fwid], F32, tag="elu_f")
            nc.vector.scalar_tensor_tensor(out=hf[:], in0=ps, scalar=nbap,
                                           in1=em[:], op0=Alu.max, op1=Alu.add)
            nc.vector.tensor_copy(hi_out, hf[:])
            nc.gpsimd.tensor_tensor(out=lo_out, in0=hf[:], in1=hi_out,
                                    op=Alu.subtract)

        # ---------- encoder over blocks ----------
        for b in range(NBT):
            r0 = b * nblk
            xh = xin.tile([128, 4 * nblk], F16, tag="xh")
            xl = xin.tile([128, 4 * nblk], F16, tag="xl")
            for kc in range(4):
                nc.sync.dma_start_transpose(
                    xh[:, kc * nblk:(kc + 1) * nblk],
                    ch_d[r0:r0 + nblk, kc * 128:(kc + 1) * 128])
                nc.sync.dma_start_transpose(
                    xl[:, kc * nblk:(kc + 1) * nblk],
                    cl_d[r0:r0 + nblk, kc * 128:(kc + 1) * 128])
            h1h = hwork.tile([128, 4 * nblk], F16, tag="h1h")
            h1l = hwork.tile([128, 4 * nblk], F16, tag="h1l")
            for ft in range(4):
                for hh in range(nblk // 1024):
                    ps = pmm.tile([128, 1024], F32, tag="pmm")
                    for sc_ in range(2):
                        o = hh * 1024 + sc_ * 512
                        po = ps[:, sc_ * 512:(sc_ + 1) * 512]
                        first = True
                        for kc in range(4):
                            wsl_h = w1c_h[:, kc * H1 + ft * 128:kc * H1 + ft * 128 + 128]
                            wsl_l = w1c_l[:, kc * H1 + ft * 128:kc * H1 + ft * 128 + 128]
                            rh = xh[:, kc * nblk + o:kc * nblk + o + 512]
                            rl = xl[:, kc * nblk + o:kc * nblk + o + 512]
                            nc.tensor.matmul(po, wsl_h, rh, start=first, stop=False)
                            first = False
                            nc.tensor.matmul(po, wsl_h, rl, start=False, stop=False)
                            nc.tensor.matmul(po, wsl_l, rh, start=False, stop=False)
                        ra_h = aT_h[:, r0 + o:r0 + o + 512]
                        ra_l = aT_l[:, r0 + o:r0 + o + 512]
                        wa_h = w1a_h[:, ft * 128:ft * 128 + 128]
                        wa_l = w1a_l[:, ft * 128:ft * 128 + 128]
                        nc.tensor.matmul(po, wa_h, ra_h, start=False, stop=False)
                        nc.tensor.matmul(po, wa_h, ra_l, start=False, stop=False)
                        nc.tensor.matmul(po, wa_l, ra_h, start=False, stop=True)
                    o = hh * 1024
                    elu_split(ps[:], 1024, b1[:, ft:ft + 1], nb1[:, ft:ft + 1],
                              h1h[:, ft * nblk + o:ft * nblk + o + 1024],
                              h1l[:, ft * nblk + o:ft * nblk + o + 1024])
            h2h = hwork.tile([128, 2 * nblk], F16, tag="h2h")
            h2l = hwork.tile([128, 2 * nblk], F16, tag="h2l")
            for ft in range(2):
                for hh in range(nblk // 1024):
                    ps = pmm.tile([128, 1024], F32, tag="pmm")
                    for sc_ in range(2):
                        o = hh * 1024 + sc_ * 512
                        po = ps[:, sc_ * 512:(sc_ + 1) * 512]
                        first = True
                        for kc in range(4):
                            wsl_h = w2_h[:, (kc * 2 + ft) * 128:(kc * 2 + ft) * 128 + 128]
                            wsl_l = w2_l[:, (kc * 2 + ft) * 128:(kc * 2 + ft) * 128 + 128]
                            rh = h1h[:, kc * nblk + o:kc * nblk + o + 512]
                            rl = h1l[:, kc * nblk + o:kc * nblk + o + 512]
                            nc.tensor.matmul(po, wsl_h, rh, start=first, stop=False)
                            first = False
                            nc.tensor.matmul(po, wsl_h, rl, start=False, stop=False)
                            nc.tensor.matmul(po, wsl_l, rh, start=False,
                                             stop=(kc == 3))
                    o = hh * 1024
                    elu_split(ps[:], 1024, b2[:, ft:ft + 1], nb2[:, ft:ft + 1],
                              h2h[:, ft * nblk + o:ft * nblk + o + 1024],
                              h2l[:, ft * nblk + o:ft * nblk + o + 1024])
            # L3 row-major: z tiles
            for tt in range(TPB):
                t = b * TPB + tt
                pz = psm.tile([128, 512], F32, tag="psm")
                pzv = pz[:, 0:EMB]
                first = True
                for kc in range(2):
                    lh_ = h2h[:, kc * nblk + tt * 128:kc * nblk + tt * 128 + 128]
                    ll_ = h2l[:, kc * nblk + tt * 128:kc * nblk + tt * 128 + 128]
                    wh_ = w3_h[:, kc * EMB:(kc + 1) * EMB]
                    wl_ = w3_l[:, kc * EMB:(kc + 1) * EMB]
                    nc.tensor.matmul(pzv, lh_, wh_, start=first, stop=False)
                    first = False
                    nc.tensor.matmul(pzv, lh_, wl_, start=False, stop=False)
                    nc.tensor.matmul(pzv, ll_, wh_, start=False, stop=False)
                nc.tensor.matmul(pzv, ones_r[:], b3_h[:], start=False, stop=False)
                nc.tensor.matmul(pzv, ones_r[:], b3_l[:], start=False, stop=True)
                nc.scalar.copy(zsb[:, t * EMB:(t + 1) * EMB], pzv)
            # normalize block -> zn hi/lo, store to DRAM
            t0 = b * TPB
            zblk = zsb[:, t0 * EMB:(t0 + TPB) * EMB]
            zsq = scr.tile([128, TPB * EMB], F32, tag="zsq")
            nc.scalar.square(zsq[:], zblk)
            nrm = scrs.tile([128, TPB], F32, tag="nrm")
            nc.vector.tensor_reduce(
                nrm[:], zsq[:].rearrange("p (t e) -> p t e", e=EMB),
                axis=mybir.AxisListType.X, op=Alu.add)
            srt = scrs.tile([128, TPB], F32, tag="srt")
            nc.scalar.activation(srt[:], nrm[:], Act.Sqrt, bias=1e-30, scale=1.0)
            s_ = scrs.tile([128, TPB], F32, tag="sinv")
            nc.vector.reciprocal(s_[:], srt[:])
            for tt in range(TPB):
                t = t0 + tt
                sl = slice(t * EMB, (t + 1) * EMB)
                nc.vector.tensor_scalar(
                    out=znh_sb[:, sl], in0=zsb[:, sl],
                    scalar1=s_[:, tt:tt + 1], scalar2=None, op0=Alu.mult)
                zf = scrs.tile([128, EMB], F32, tag="znf")
                nc.vector.tensor_scalar(
                    out=zf[:], in0=zsb[:, sl],
                    scalar1=s_[:, tt:tt + 1], scalar2=None, op0=Alu.mult)
                nc.vector.tensor_tensor(out=znl_sb[:, sl], in0=zf[:],
                                        in1=znh_sb[:, sl], op=Alu.subtract)
            nc.sync.dma_start(
                out=znh_d[r0:r0 + nblk, :].rearrange("(t p) e -> p (t e)", p=128),
                in_=znh_sb[:, t0 * EMB:(t0 + TPB) * EMB])
            nc.sync.dma_start(
                out=znl_d[r0:r0 + nblk, :].rearrange("(t p) e -> p (t e)", p=128),
                in_=znl_sb[:, t0 * EMB:(t0 + TPB) * EMB])

        # ---------- allgather zn ----------
        nc.gpsimd.collective_compute(
            "AllGather", Alu.bypass, replica_groups=groups,
            ins=[znh_d.opt()], outs=[zgh_d.opt()])
        nc.gpsimd.collective_compute(
            "AllGather", Alu.bypass, replica_groups=groups,
            ins=[znl_d.opt()], outs=[zgl_d.opt()])

        znT_h = rtile([EMB, ns], F16, "znTh")
        znT_l = rtile([EMB, ns], F16, "znTl")
        nc.sync.dma_start_transpose(znT_h[:], znh_d[:])
        nc.sync.dma_start_transpose(znT_l[:], znl_d[:])
        zgT_h = rtile([EMB, NFULL], F16, "zgTh")
        zgT_l = rtile([EMB, NFULL], F16, "zgTl")
        nc.sync.dma_start_transpose(zgT_h[:], zgh_d[:])
        nc.sync.dma_start_transpose(zgT_l[:], zgl_d[:])

        # ---------- Dt: column-sharded distances [128, LH] ----------
        for j in range(LH // 512):
            pd = psm.tile([128, 512], F32, tag="psm")
            for half in range(2):
                jj = j + half * (LH // 512)
                rh = zgT_h[:, jj * 512:(jj + 1) * 512]
                rl = zgT_l[:, jj * 512:(jj + 1) * 512]
                po = pd[half * 64:(half + 1) * 64, :]
                tp = (0, 64) if half else None
                nc.tensor.matmul(po, ensl_h[:], rh, start=True, stop=False,
                                 tile_position=tp)
                nc.tensor.matmul(po, ensl_h[:], rl, start=False, stop=False,
                                 tile_position=tp)
                nc.tensor.matmul(po, ensl_l[:], rh, start=False, stop=True,
                                 tile_position=tp)
            nc.scalar.copy(Dt[:, j * 512:(j + 1) * 512], pd[:])

        # ---------- row-major D: argmax, onehot, counts, idx ----------
        pk = pk1.tile([1, K], F32, tag="counts")
        for t in range(NT):
            pD = psm.tile([128, 512], F32, tag="psm")
            lh_ = znT_h[:, t * 128:(t + 1) * 128]
            ll_ = znT_l[:, t * 128:(t + 1) * 128]
            nc.tensor.matmul(pD[:], lh_, ent_h[:], start=True, stop=False)
            nc.tensor.matmul(pD[:], lh_, ent_l[:], start=False, stop=False)
            nc.tensor.matmul(pD[:], ll_, ent_h[:], start=False, stop=True)
            rmax = scrs.tile([128, 1], F32, tag="rmax")
            nc.vector.tensor_reduce(rmax[:], pD[:], axis=mybir.AxisListType.X,
                                    op=Alu.max)
            oh_t = oh_full[:, t * K:(t + 1) * K]
            nc.vector.tensor_scalar(out=oh_t, in0=pD[:], scalar1=rmax[:],
                                    scalar2=None, op0=Alu.is_ge)
            nc.tensor.matmul(pk[:], ones_c[:], oh_t, start=(t == 0),
                             stop=(t == NT - 1))
            sidx = scr.tile([128, K], F32, tag="sidx")
            nc.vector.scalar_tensor_tensor(
                out=sidx[:], in0=oh_t, scalar=1.0, in1=iota[:],
                op0=Alu.mult, op1=Alu.mult,
                accum_out=idx_sb[:, t:t + 1])
            nc.sync.dma_start(out=oh_d[t * 128:(t + 1) * 128, :], in_=oh_t)
        idx_i = scrs.tile([128, NT], I32, tag="idxi")
        nc.vector.tensor_copy(idx_i[:], idx_sb[:])
        nc.sync.dma_start(out=idx_d.rearrange("(t p) -> p t", p=128),
                          in_=idx_i[:])

        # ---------- q = onehot @ E (via transposed onehot chunks) ----------
        for b in range(NBT):
            r0 = b * nblk
            ohT = xin.tile([128, 4 * nblk], F16, tag="ohT")
            for kc in range(4):
                nc.sync.dma_start_transpose(
                    ohT[:, kc * nblk:(kc + 1) * nblk],
                    oh_d[r0:r0 + nblk, kc * 128:(kc + 1) * 128])
            for tt in range(TPB):
                t = b * TPB + tt
                pq = psm.tile([128, 512], F32, tag="psm")
                pqv = pq[:, 0:EMB]
                first = True
                for kc in range(4):
                    osl = ohT[:, kc * nblk + tt * 128:kc * nblk + tt * 128 + 128]
                    nc.tensor.matmul(pqv, osl, e_h[:, kc * EMB:(kc + 1) * EMB],
                                     start=first, stop=False)
                    first = False
                    nc.tensor.matmul(pqv, osl, e_l[:, kc * EMB:(kc + 1) * EMB],
                                     start=False, stop=(kc == 3))
                nc.scalar.copy(q_sb[:, t * EMB:(t + 1) * EMB], pqv)
                nc.vector.tensor_copy(q16_sb[:, t * EMB:(t + 1) * EMB], pqv)
        nc.sync.dma_start(out=qst_d.rearrange("(t p) e -> p (t e)", p=128),
                          in_=q_sb[:])
        nc.sync.dma_start(out=q16_d.rearrange("(t p) e -> p (t e)", p=128),
                          in_=q16_sb[:])

        # ---------- contra stats on Dt ----------
        # segment top-8 candidates
        cand0 = rtile([128, NSEG * 8], F16, "cand0")
        for s in range(NSEG):
            nc.vector.max(cand0[:, s * 8:(s + 1) * 8],
                          Dt[:, s * SEG:(s + 1) * SEG])
        cand = rtile([64, CW], F16, "cand")
        nc.sync.dma_start(out=cand[:, 0:NSEG * 8], in_=cand0[0:64, :])
        nc.sync.dma_start(out=cand[:, NSEG * 8:CW], in_=cand0[64:128, :])

        # bisect for top-NPOS threshold on candidates
        tlo = rtile([64, 1], F32, "tlo")
        thi = rtile([64, 1], F32, "thi")
        nc.vector.memset(tlo[:], -1.0)
        nc.vector.memset(thi[:], 1.0)
        tmid = rtile([64, 1], F32, "tmid")
        for it in range(15):
            nc.vector.tensor_tensor(out=tmid[:], in0=tlo[:], in1=thi[:],
                                    op=Alu.add)
            nc.vector.tensor_scalar_mul(tmid[:], tmid[:], 0.5)
            scc = scrs.tile([64, CW], F16, tag="scc")
            cntc = scrs.tile([64, 1], F32, tag="cntc")
            nc.vector.tensor_scalar(out=scc[:], in0=cand[:], scalar1=tmid[:],
                                    scalar2=None, op0=Alu.is_gt,
                                    accum_out=cntc[:])
            sel = scrs.tile([64, 1], F32, tag="selc")
            nc.vector.tensor_scalar(out=sel[:], in0=cntc[:],
                                    scalar1=float(NPOS), scalar2=None,
                                    op0=Alu.is_ge)
            nc.vector.select(tlo[:], sel[:], tmid[:], tlo[:])
            nc.vector.select(thi[:], sel[:], thi[:], tmid[:])
        t128 = rtile([128, 1], F32, "t128")
        nc.vector.tensor_copy(t128[0:64, :], tlo[:])
        nc.sync.dma_start(out=t128[64:128, :], in_=tlo[:])

        # p = (sum relu(v - t) + NPOS*t)/NPOS   (8 chunks, ACT relu-accum)
        negt = rtile([128, 1], F32, "negt")
        nc.vector.tensor_scalar_mul(negt[:], t128[:], -1.0)
        pacc = rtile([128, NCHUNK], F32, "pacc")
        for cchunk in range(NCHUNK):
            so = scr.tile([128, CH], F16, tag="prelu")
            nc.scalar.activation(so[:], Dt[:, cchunk * CH:(cchunk + 1) * CH],
                                 Act.Relu, bias=negt[:], scale=1.0,
                                 accum_out=pacc[:, cchunk:cchunk + 1])
        psum_ = rtile([128, 1], F32, "psum_")
        nc.vector.tensor_reduce(psum_[:], pacc[:], axis=mybir.AxisListType.X,
                                op=Alu.add)
        pval = rtile([128, 1], F32, "pval")
        nc.vector.tensor_scalar(out=pval[:], in0=psum_[:],
                                scalar1=1.0 / NPOS, scalar2=t128[:],
                                op0=Alu.mult, op1=Alu.add)

        # median bisect on strided subsample (merged count across halves)
        mlo = rtile([64, 1], F32, "mlo")
        mhi = rtile([64, 1], F32, "mhi")
        nc.vector.memset(mlo[:], -1.0)
        nc.vector.memset(mhi[:], 1.0)
        mmid = rtile([128, 1], F32, "mmid")
        sub_ap = Dt[:, 0:LH:SST]
        for it in range(16):
            nc.vector.tensor_tensor(out=mmid[0:64, :], in0=mlo[:], in1=mhi[:],
                                    op=Alu.add)
            nc.vector.tensor_scalar_mul(mmid[0:64, :], mmid[0:64, :], 0.5)
            nc.sync.dma_start(out=mmid[64:128, :], in_=mmid[0:64, :])
            scm = scr.tile([128, SUBW], F16, tag="scm")
            cntm = scrs.tile([128, 1], F32, tag="cntm")
            nc.vector.tensor_scalar(out=scm[:], in0=sub_ap, scalar1=mmid[:],
                                    scalar2=None, op0=Alu.is_lt,
                                    accum_out=cntm[:])
            cnt2 = scrs.tile([64, 1], F32, tag="cnt2")
            nc.sync.dma_start(out=cnt2[:], in_=cntm[64:128, :])
            nc.vector.tensor_tensor(out=cnt2[:], in0=cnt2[:], in1=cntm[0:64, :],
                                    op=Alu.add)
            selm = scrs.tile([64, 1], F32, tag="selm")
            nc.vector.tensor_scalar(out=selm[:], in0=cnt2[:],
                                    scalar1=float(2 * SUBW // 2), scalar2=None,
                                    op0=Alu.is_lt)
            nc.vector.select(mlo[:], selm[:], mmid[0:64, :], mlo[:])
            nc.vector.select(mhi[:], selm[:], mhi[:], mmid[0:64, :])
        mfin = rtile([128, 1], F32, "mfin")
        nc.vector.tensor_tensor(out=mfin[0:64, :], in0=mlo[:], in1=mhi[:],
                                op=Alu.add)
        nc.vector.tensor_scalar_mul(mfin[0:64, :], mfin[0:64, :], 0.5)
        nc.sync.dma_start(out=mfin[64:128, :], in_=mfin[0:64, :])

        # exp pass + masked sums: U = sum_{v<m} exp((v-m)/tau - 15), cnt_less
        ESH = 15.0
        bm = rtile([128, 1], F32, "bm")
        nc.vector.tensor_scalar(out=bm[:], in0=mfin[:], scalar1=-1.0 / 0.07,
                                scalar2=-ESH, op0=Alu.mult, op1=Alu.add)
        uacc = rtile([128, NCHUNK], F32, "uacc")
        cacc = rtile([128, NCHUNK], F32, "cacc")
        for cchunk in range(NCHUNK):
            dsl = Dt[:, cchunk * CH:(cchunk + 1) * CH]
            ech = scr.tile([128, CH], F16, tag="ech")
            nc.scalar.activation(ech[:], dsl, Act.Exp, bias=bm[:],
                                 scale=1.0 / 0.07)
            mch = scr.tile([128, CH], F16, tag="mch")
            nc.vector.scalar_tensor_tensor(
                out=mch[:], in0=dsl, scalar=mfin[:], in1=ech[:],
                op0=Alu.is_lt, op1=Alu.mult,
                accum_out=uacc[:, cchunk:cchunk + 1])
            sch = scr.tile([128, CH], F16, tag="sch")
            nc.vector.tensor_scalar(out=sch[:], in0=dsl, scalar1=mfin[:],
                                    scalar2=None, op0=Alu.is_lt,
                                    accum_out=cacc[:, cchunk:cchunk + 1])
        u1 = rtile([128, 1], F32, "u1")
        c1 = rtile([128, 1], F32, "c1")
        nc.vector.tensor_reduce(u1[:], uacc[:], axis=mybir.AxisListType.X,
                                op=Alu.add)
        nc.vector.tensor_reduce(c1[:], cacc[:], axis=mybir.AxisListType.X,
                                op=Alu.add)
        u2 = rtile([64, 1], F32, "u2")
        c2 = rtile([64, 1], F32, "c2")
        nc.sync.dma_start(out=u2[:], in_=u1[64:128, :])
        nc.sync.dma_start(out=c2[:], in_=c1[64:128, :])
        nc.vector.tensor_tensor(out=u2[:], in0=u2[:], in1=u1[0:64, :], op=Alu.add)
        nc.vector.tensor_tensor(out=c2[:], in0=c2[:], in1=c1[0:64, :], op=Alu.add)
        # T = (U*e^15 + (N/2 - cnt)) * exp((m-p)/tau); ck = log1p(T)
        w2t = rtile([64, 1], F32, "w2t")
        nc.vector.tensor_scalar(out=w2t[:], in0=c2[:], scalar1=-1.0,
                                scalar2=float(NFULL // 2), op0=Alu.mult,
                                op1=Alu.add)
        u3 = rtile([64, 1], F32, "u3")
        nc.vector.tensor_scalar_mul(u3[:], u2[:], float(np.exp(ESH)))
        t0_ = rtile([64, 1], F32, "t0_")
        nc.vector.tensor_tensor(out=t0_[:], in0=u3[:], in1=w2t[:], op=Alu.add)
        dmp = rtile([64, 1], F32, "dmp")
        nc.vector.tensor_tensor(out=dmp[:], in0=mfin[0:64, :], in1=pval[0:64, :],
                                op=Alu.subtract)
        g_ = rtile([64, 1], F32, "g_")
        nc.scalar.activation(g_[:], dmp[:], Act.Exp, bias=0.0, scale=1.0 / 0.07)
        tv = rtile([64, 1], F32, "tv")
        nc.vector.tensor_tensor(out=tv[:], in0=t0_[:], in1=g_[:], op=Alu.mult)
        ck = rtile([64, 1], F32, "ck")
        nc.scalar.activation(ck[:], tv[:], Act.Ln, bias=1.0, scale=1.0)
        pc = psm.tile([128, 512], F32, tag="psm")
        nc.tensor.matmul(pc[0:1, 0:1], ck[:], ones_c32[:], start=True, stop=True)

        # ---------- decoder ----------
        for b in range(NBT):
            r0 = b * nblk
            xh = xin.tile([128, 4 * nblk], F16, tag="xh")
            xl = xin.tile([128, 4 * nblk], F16, tag="xl")
            for kc in range(4):
                nc.sync.dma_start_transpose(
                    xh[:, kc * nblk:(kc + 1) * nblk],
                    ch_d[r0:r0 + nblk, kc * 128:(kc + 1) * 128])
                nc.sync.dma_start_transpose(
                    xl[:, kc * nblk:(kc + 1) * nblk],
                    cl_d[r0:r0 + nblk, kc * 128:(kc + 1) * 128])
            qT = xin.tile([EMB, nblk], F16, tag="qT")
            nc.sync.dma_start_transpose(qT[:], q16_d[r0:r0 + nblk, :])
            hdh = hwork.tile([128, 2 * nblk], F16, tag="hdh")
            hdl = hwork.tile([128, 2 * nblk], F16, tag="hdl")
            for ft in range(2):
                for hh in range(nblk // 1024):
                    ps = pmm.tile([128, 1024], F32, tag="pmm")
                    for sc_ in range(2):
                        o = hh * 1024 + sc_ * 512
                        po = ps[:, sc_ * 512:(sc_ + 1) * 512]
                        first = True
                        for kc in range(4):
                            wh_ = wd1c_h[:, (kc * 2 + ft) * 128:(kc * 2 + ft) * 128 + 128]
                            wl_ = wd1c_l[:, (kc * 2 + ft) * 128:(kc * 2 + ft) * 128 + 128]
                            rh = xh[:, kc * nblk + o:kc * nblk + o + 512]
                            rl = xl[:, kc * nblk + o:kc * nblk + o + 512]
                            nc.tensor.matmul(po, wh_, rh, start=first, stop=False)
                            first = False
                            nc.tensor.matmul(po, wh_, rl, start=False, stop=False)
                            nc.tensor.matmul(po, wl_, rh, start=False, stop=False)
                        nc.tensor.matmul(po, wd1q[:, ft * 128:ft * 128 + 128],
                                         qT[:, o:o + 512], start=False, stop=True)
                    o = hh * 1024
                    elu_split(ps[:], 1024, bd1[:, ft:ft + 1], nbd1[:, ft:ft + 1],
                              hdh[:, ft * nblk + o:ft * nblk + o + 1024],
                              hdl[:, ft * nblk + o:ft * nblk + o + 1024])
            h2dh = hwork.tile([128, 4 * nblk], F16, tag="h2dh")
            h2dl = hwork.tile([128, 4 * nblk], F16, tag="h2dl")
            for ft in range(4):
                for hh in range(nblk // 1024):
                    ps = pmm.tile([128, 1024], F32, tag="pmm")
                    for sc_ in range(2):
                        o = hh * 1024 + sc_ * 512
                        po = ps[:, sc_ * 512:(sc_ + 1) * 512]
                        first = True
                        for kc in range(2):
                            wh_ = wd2_h[:, (kc * 4 + ft) * 128:(kc * 4 + ft) * 128 + 128]
                            wl_ = wd2_l[:, (kc * 4 + ft) * 128:(kc * 4 + ft) * 128 + 128]
                            rh = hdh[:, kc * nblk + o:kc * nblk + o + 512]
                            rl = hdl[:, kc * nblk + o:kc * nblk + o + 512]
                            nc.tensor.matmul(po, wh_, rh, start=first, stop=False)
                            first = False
                            nc.tensor.matmul(po, wh_, rl, start=False, stop=False)
                            nc.tensor.matmul(po, wl_, rh, start=False,
                                             stop=(kc == 1))
                    o = hh * 1024
                    elu_split(ps[:], 1024, bd2[:, ft:ft + 1], nbd2[:, ft:ft + 1],
                              h2dh[:, ft * nblk + o:ft * nblk + o + 1024],
                              h2dl[:, ft * nblk + o:ft * nblk + o + 1024])
            for tt in range(TPB):
                t = b * TPB + tt
                pr = psm.tile([128, 512], F32, tag="psm")
                prv = pr[:, 0:ACTD]
                first = True
                for kc in range(4):
                    lh_ = h2dh[:, kc * nblk + tt * 128:kc * nblk + tt * 128 + 128]
                    ll_ = h2dl[:, kc * nblk + tt * 128:kc * nblk + tt * 128 + 128]
                    wh_ = wd3_h[:, kc * ACTD:(kc + 1) * ACTD]
                    wl_ = wd3_l[:, kc * ACTD:(kc + 1) * ACTD]
                    nc.tensor.matmul(prv, lh_, wh_, start=first, stop=False)
                    first = False
                    nc.tensor.matmul(prv, lh_, wl_, start=False, stop=False)
                    nc.tensor.matmul(prv, ll_, wh_, start=False, stop=False)
                nc.tensor.matmul(prv, ones_r[:], bd3_h[:], start=False, stop=False)
                nc.tensor.matmul(prv, ones_r[:], bd3_l[:], start=False, stop=True)
                nc.scalar.copy(recon_sb[:, t * ACTD:(t + 1) * ACTD], prv)
        nc.sync.dma_start(out=recon_d.rearrange("(t p) e -> p (t e)", p=128),
                          in_=recon_sb[:])

        # ---------- losses ----------
        dql = scr.tile([128, NT * EMB], F32, tag="dql")
        nc.vector.tensor_tensor(out=dql[:], in0=q_sb[:], in1=zsb[:],
                                op=Alu.subtract)
        sq1 = rtile([128, 1], F32, "sq1")
        dqs = scr.tile([128, NT * EMB], F32, tag="dqs")
        nc.scalar.activation(dqs[:], dql[:], Act.Square, bias=0.0, scale=1.0,
                             accum_out=sq1[:])
        ps_s = psm.tile([128, 512], F32, tag="psm")
        one128 = rtile([128, 1], F32, "one128")
        nc.vector.memset(one128[:], 1.0)
        nc.tensor.matmul(ps_s[0:1, 0:1], sq1[:], one128[:], start=True, stop=True)

        drl = scr.tile([128, NT * ACTD], F32, tag="drl")
        nc.vector.tensor_tensor(out=drl[:], in0=recon_sb[:], in1=act_sb[:],
                                op=Alu.subtract)
        sr1 = rtile([128, 1], F32, "sr1")
        drs = scr.tile([128, NT * ACTD], F32, tag="drs")
        nc.scalar.activation(drs[:], drl[:], Act.Square, bias=0.0, scale=1.0,
                             accum_out=sr1[:])
        ps_r = psm.tile([128, 512], F32, tag="psm")
        nc.tensor.matmul(ps_r[0:1, 0:1], sr1[:], one128[:], start=True, stop=True)

        # ---------- pack scalars, allreduce, finalize ----------
        sci = rtile([1, 520], F32, "sci")
        nc.vector.tensor_copy(sci[:, 0:1], ps_s[0:1, 0:1])
        nc.vector.tensor_copy(sci[:, 1:2], ps_r[0:1, 0:1])
        nc.vector.tensor_copy(sci[:, 2:3], pc[0:1, 0:1])
        nc.vector.tensor_copy(sci[:, 8:8 + K], pk[:])
        nc.sync.dma_start(out=sci_d[:], in_=sci[:])
        nc.gpsimd.collective_compute(
            "AllReduce", Alu.add, replica_groups=groups,
            ins=[sci_d.opt()], outs=[sco_d.opt()])
        sco = rtile([1, 520], F32, "sco")
        nc.sync.dma_start(out=sco[:], in_=sco_d[:])

        scal = rtile([1, 8], F32, "scal")
        # q_latent = S/(N*EMB); e_latent = CC*q_latent; recon = R/(N*ACT); contra/K
        nc.vector.tensor_scalar_mul(scal[:, 0:1], sco[:, 0:1],
                                    1.0 / (NFULL * EMB))
        nc.vector.tensor_scalar_mul(scal[:, 1:2], sco[:, 0:1],
                                    CC / (NFULL * EMB))
        nc.vector.tensor_scalar_mul(scal[:, 2:3], sco[:, 2:3], 1.0 / K)
        nc.vector.tensor_scalar_mul(scal[:, 4:5], sco[:, 1:2],
                                    1.0 / (NFULL * ACTD))
        # perplexity from counts
        pr_ = scrs.tile([1, K], F32, tag="pr_")
        nc.vector.tensor_scalar_mul(pr_[:], sco[:, 8:8 + K], 1.0 / NFULL)
        lg_ = scrs.tile([1, K], F32, tag="lg_")
        nc.scalar.activation(lg_[:], pr_[:], Act.Ln, bias=1e-10, scale=1.0)
        pl_ = scrs.tile([1, K], F32, tag="pl_")
        ent_acc = scrs.tile([1, 1], F32, tag="entacc")
        nc.vector.tensor_tensor(out=pl_[:], in0=pr_[:], in1=lg_[:], op=Alu.mult)
        nc.vector.tensor_reduce(ent_acc[:], pl_[:], axis=mybir.AxisListType.X,
                                op=Alu.add)
        nc.scalar.activation(scal[:, 3:4], ent_acc[:], Act.Exp, bias=0.0,
                             scale=-1.0)
        nc.sync.dma_start(out=sc_d[:], in_=scal[:])

        for p in (res, xin, hwork, scr, scrs, pmm, psm, pk1):
            p.__exit__(None, None, None)

    nc.compile()
    return nc


def host_prep(We1, be1, We2, be2, We3, be3, Wd1, bd1, Wd2, bd2, Wd3, bd3, E):
    """Host-side weight packing (f16 hi/lo splits, bias folds, transposes)."""
    o = {}
    w1a = We1[0:ACTD]                      # [12, 512]
    w1c = We1[ACTD:ACTD + COND]            # [512, 512]
    w1c_pack = np.concatenate([w1c[kc * 128:(kc + 1) * 128] for kc in range(4)],
                              axis=1)      # [128, 2048]
    o["w1c_h"], o["w1c_l"] = f16(w1c_pack), f16lo(w1c_pack)
    o["w1a_h"], o["w1a_l"] = f16(w1a), f16lo(w1a)
    o["b1"] = be1.reshape(4, 128).T.astype(np.float32).copy()
    o["nb1"] = -o["b1"]
    w2p = np.concatenate(
        [We2[kc * 128:(kc + 1) * 128, ft * 128:(ft + 1) * 128]
         for kc in range(4) for ft in range(2)], axis=1)
    o["w2_h"], o["w2_l"] = f16(w2p), f16lo(w2p)
    b2f = (be2 - We2.sum(0)).astype(np.float32)
    o["b2"] = b2f.reshape(2, 128).T.copy()
    o["nb2"] = -o["b2"]
    w3p = np.concatenate([We3[kc * 128:(kc + 1) * 128] for kc in range(2)],
                         axis=1)
    o["w3_h"], o["w3_l"] = f16(w3p), f16lo(w3p)
    b3f = (be3 - We3.sum(0)).astype(np.float32).reshape(1, EMB)
    o["b3_h"], o["b3_l"] = f16(b3f), f16lo(b3f)
    En = E / np.maximum(np.linalg.norm(E, axis=-1, keepdims=True), 1e-12)
    EnT = En.T.astype(np.float32)          # [16, 512]
    o["ent_h"], o["ent_l"] = f16(EnT), f16lo(EnT)
    ep = np.concatenate([E[kc * 128:(kc + 1) * 128] for kc in range(4)], axis=1)
    o["e_h"], o["e_l"] = f16(ep), f16lo(ep)
    wd1q = Wd1[0:EMB]                      # [16, 256]
    wd1c = Wd1[EMB:EMB + COND]             # [512, 256]
    wd1p = np.concatenate(
        [wd1c[kc * 128:(kc + 1) * 128, ft * 128:(ft + 1) * 128]
         for kc in range(4) for ft in range(2)], axis=1)
    o["wd1c_h"], o["wd1c_l"] = f16(wd1p), f16lo(wd1p)
    o["wd1q"] = f16(wd1q)
    o["bd1"] = bd1.reshape(2, 128).T.astype(np.float32).copy()
    o["nbd1"] = -o["bd1"]
    wd2p = np.concatenate(
        [Wd2[kc * 128:(kc + 1) * 128, ft * 128:(ft + 1) * 128]
         for kc in range(2) for ft in range(4)], axis=1)
    o["wd2_h"], o["wd2_l"] = f16(wd2p), f16lo(wd2p)
    bd2f = (bd2 - Wd2.sum(0)).astype(np.float32)
    o["bd2"] = bd2f.reshape(4, 128).T.copy()
    o["nbd2"] = -o["bd2"]
    wd3p = np.concatenate([Wd3[kc * 128:(kc + 1) * 128] for kc in range(4)],
                          axis=1)
    o["wd3_h"], o["wd3_l"] = f16(wd3p), f16lo(wd3p)
    bd3f = (bd3 - Wd3.sum(0)).astype(np.float32).reshape(1, ACTD)
    o["bd3_h"], o["bd3_l"] = f16(bd3f), f16lo(bd3f)
    o["iota"] = np.broadcast_to(np.arange(K, dtype=np.float32), (128, K)).copy()
    o["_EnT"] = EnT
    return o


def make_in_maps(actions, conditions, wp, ns):
    maps = []
    EnT = wp["_EnT"]
    shared = {k: v for k, v in wp.items() if not k.startswith("_")}
    for c in range(NCORE):
        sl = slice(c * ns, (c + 1) * ns)
        a = np.asarray(actions[sl], np.float32)
        cd = np.asarray(conditions[sl], np.float32)
        m = dict(shared)
        m["a32"] = a
        m["ah"], m["al"] = f16(a), f16lo(a)
        m["ch"], m["cl"] = f16(cd), f16lo(cd)
        esl = EnT[:, c * 64:(c + 1) * 64]
        m["ensl_h"], m["ensl_l"] = f16(esl), f16lo(esl)
        maps.append(m)
    return maps


_NC_CACHE = {}


def _get_nc(ns, nblk):
    key = (ns, nblk)
    if key not in _NC_CACHE:
        _NC_CACHE[key] = build(ns, nblk)
    return _NC_CACHE[key]


def kernel(actions, conditions, We1, be1, We2, be2, We3, be3,
           Wd1, bd1, Wd2, bd2, Wd3, bd3, E, _trace=False):
    from concourse.bass_utils import run_bass_kernel_spmd
    ns = actions.shape[0] // NCORE
    nblk = min(2048, ns)
    nc = _get_nc(ns, nblk)
    wp = host_prep(np.asarray(We1, np.float32), np.asarray(be1, np.float32),
                   np.asarray(We2, np.float32), np.asarray(be2, np.float32),
                   np.asarray(We3, np.float32), np.asarray(be3, np.float32),
                   np.asarray(Wd1, np.float32), np.asarray(bd1, np.float32),
                   np.asarray(Wd2, np.float32), np.asarray(bd2, np.float32),
                   np.asarray(Wd3, np.float32), np.asarray(bd3, np.float32),
                   np.asarray(E, np.float32))
    maps = make_in_maps(actions, conditions, wp, ns)
    res = run_bass_kernel_spmd(nc, maps, core_ids=list(range(NCORE)),
                               trace=_trace)
    r = res.results
    recon = np.concatenate([r[c]["recon"] for c in range(NCORE)], axis=0)
    q_st = np.concatenate([r[c]["qst"] for c in range(NCORE)], axis=0)
    idx = np.concatenate([r[c]["idx"] for c in range(NCORE)], axis=0)
    sc = r[0]["scal"]
    out = (recon.astype(np.float32), q_st.astype(np.float32),
           idx.astype(np.int32),
           np.float32(sc[0, 0]), np.float32(sc[0, 1]), np.float32(sc[0, 2]),
           np.float32(sc[0, 3]), np.float32(sc[0, 4]))
    if _trace:
        return out, res
    return out


# revision 72
# speedup vs baseline: 1.2572x; 1.2572x over previous
"""Self-contained Trainium2 kernel for nn_ActionQuantizer (VQ-VAE action quantizer).

Strategy: data-parallel over the N=65536 batch across 8 NeuronCores.
- Encoder/decoder MLPs run feature-major with fp16 hi/lo x3 matmuls (~fp32 precision).
- VQ distances computed twice: row-major (argmax/onehot/q) and column-sharded
  (64 codebook columns per core over the full batch, via an AllGather of zn)
  for the contrastive-loss order statistics.
- Contrastive sort stats via threshold selection: segment-top8 candidates ->
  bisect for the top-n_pos threshold; strided-subsample bisect for the median;
  exact correction terms make the result insensitive to small threshold error.
- Scalar losses all-reduced on device; host only reassembles shards.
"""
import sys

sys.path.insert(0, "/opt/trn_rl_repo")
import numpy as np
import ml_dtypes  # noqa: F401

N_TOT = 65536
_PHASES = []
NCORE = 8
ACTD = 12
COND = 512
H1 = 512
H2 = 256
EMB = 16
K = 512
CC = 0.25


def f16(x):
    return np.asarray(x, np.float32).astype(np.float16)


def f16lo(x):
    x = np.asarray(x, np.float32)
    return (x - x.astype(np.float16).astype(np.float32)).astype(np.float16)


def build(ns, nblk, _null=False, _tlsim=False):
    """Build the SPMD Bass program for ns rows per core, streaming block nblk."""
    import concourse.bass as bass
    import concourse.tile as tile
    from concourse import bacc, mybir
    from concourse import bass_isa

    F32 = mybir.dt.float32
    F16 = mybir.dt.float16
    I32 = mybir.dt.int32
    U8 = mybir.dt.uint8
    Alu = mybir.AluOpType
    Act = mybir.ActivationFunctionType

    NT = ns // 128            # row tiles per core
    NBT = ns // nblk          # blocks per core
    TPB = nblk // 128         # row tiles per block
    LH = 4 * ns               # per-partition half-column length of Dt (= N/2)
    SEG = min(1024, LH)
    NSEG = LH // SEG
    CW = 2 * NSEG * 8         # candidate width after halves merge
    NPOS = (NCORE * ns) // K  # top-n_pos for dis_pos
    SST = 16                  # median subsample stride
    SUBW = LH // SST
    PF = min(1024, nblk)      # psum chunk width for MLP layers
    NCHUNK = 32               # chunks for full-Dt passes
    CH = LH // NCHUNK
    NGP = min(4, ns // nblk)  # partial-gather parts
    PW = ns // NGP
    ZCH = min(1024, PW)       # zn column-chunk rows for streamed loads
    NFULL = NCORE * ns

    nc = bacc.Bacc("TRN2", target_bir_lowering=False, debug=False,
                   num_devices=1 if _tlsim else NCORE)

    def din(name, shape, dt=F32):
        return nc.dram_tensor(name, shape, dt, kind="ExternalInput").ap()

    def dout(name, shape, dt=F32):
        return nc.dram_tensor(name, shape, dt, kind="ExternalOutput").ap()

    # ---- inputs ----
    a32_d = din("a32", [ns, ACTD])
    ah_d = din("ah", [ns, ACTD], F16)
    al_d = din("al", [ns, ACTD], F16)
    ch_d = din("ch", [ns, COND], F16)
    cl_d = din("cl", [ns, COND], F16)
    w1c_h_d = din("w1c_h", [128, 4 * H1], F16)
    w1c_l_d = din("w1c_l", [128, 4 * H1], F16)
    w1a_h_d = din("w1a_h", [ACTD, H1], F16)
    w1a_l_d = din("w1a_l", [ACTD, H1], F16)
    b1_d = din("b1", [128, 4])
    nb1_d = din("nb1", [128, 4])
    w2_h_d = din("w2_h", [128, 8 * 128], F16)
    w2_l_d = din("w2_l", [128, 8 * 128], F16)
    b2_d = din("b2", [128, 2])
    nb2_d = din("nb2", [128, 2])
    w3_h_d = din("w3_h", [128, 2 * EMB], F16)
    w3_l_d = din("w3_l", [128, 2 * EMB], F16)
    b3_h_d = din("b3_h", [1, EMB], F16)
    b3_l_d = din("b3_l", [1, EMB], F16)
    ent_h_d = din("ent_h", [EMB, K], F16)
    ent_l_d = din("ent_l", [EMB, K], F16)
    ensl_h_d = din("ensl_h", [EMB, 64], F16)   # per-core column slice of EnT
    ensl_l_d = din("ensl_l", [EMB, 64], F16)
    e_h_d = din("e_h", [128, 4 * EMB], F16)
    e_l_d = din("e_l", [128, 4 * EMB], F16)
    wd1c_h_d = din("wd1c_h", [128, 8 * 128], F16)
    wd1c_l_d = din("wd1c_l", [128, 8 * 128], F16)
    wd1q_d = din("wd1q", [EMB, H2], F16)
    bd1_d = din("bd1", [128, 2])
    nbd1_d = din("nbd1", [128, 2])
    wd2_h_d = din("wd2_h", [128, 8 * 128], F16)
    wd2_l_d = din("wd2_l", [128, 8 * 128], F16)
    bd2_d = din("bd2", [128, 4])
    nbd2_d = din("nbd2", [128, 4])
    wd3_h_d = din("wd3_h", [128, 4 * ACTD], F16)
    wd3_l_d = din("wd3_l", [128, 4 * ACTD], F16)
    bd3_h_d = din("bd3_h", [1, ACTD], F16)
    bd3_l_d = din("bd3_l", [1, ACTD], F16)
    iota_d = din("iota", [128, K])
    ident_d = din("ident", [128, 128], F16)

    # ---- outputs ----
    dbg_d = dout("dbg", [128, 16])
    recon_d = dout("recon", [ns, ACTD])
    qst_d = dout("qst", [ns, EMB])
    idx_d = dout("idx", [ns], I32)
    sc_d = dout("scal", [1, 8])

    # ---- internal DRAM ----
    znh_p = [nc.dram_tensor(f"znh_b{p}", [EMB, PW], F16).ap() for p in range(NGP)]
    znl_p = [nc.dram_tensor(f"znl_b{p}", [EMB, PW], F16).ap() for p in range(NGP)]
    zgh_p = [nc.dram_tensor(f"zgh_b{p}", [NCORE * EMB, PW], F16).ap() for p in range(NGP)]
    zgl_p = [nc.dram_tensor(f"zgl_b{p}", [NCORE * EMB, PW], F16).ap() for p in range(NGP)]
    oh_d = nc.dram_tensor("oh_b", [ns, K], F16).ap()
    q16_d = nc.dram_tensor("q16_b", [EMB, ns], F16).ap()
    sci_d = nc.dram_tensor("sci_b", [1, 520], F32).ap()
    sco_d = nc.dram_tensor("sco_b", [1, 520], F32).ap()

    groups = [list(range(NCORE))]

    if _null:
        with tile.TileContext(nc) as tc:
            with tc.tile_pool(name="np_", bufs=1) as pool:
                t = pool.tile([1, 8], F32, name="nulltile")
                nc.sync.dma_start(out=t[:, 0:4], in_=b1_d[0:1, 0:4])
                nc.vector.tensor_scalar_mul(t[:], t[:], 1.0)
                nc.sync.dma_start(out=sc_d[:], in_=t[:])
                t2 = pool.tile([128, 16], F32, name="nulltile2")
                nc.vector.memset(t2[:], 0.0)
                nc.sync.dma_start(out=dbg_d[:], in_=t2[:])
                nc.sync.dma_start(out=recon_d[0:1, :], in_=t2[0:1, 0:ACTD])
                nc.sync.dma_start(out=qst_d[0:1, :], in_=t2[0:1, 0:EMB])
                it = pool.tile([1, 8], I32, name="nulltile3")
                nc.vector.memset(it[:], 0)
                nc.sync.dma_start(out=idx_d[0:8], in_=it[0, :])
        nc.compile()
        return nc

    from contextlib import ExitStack
    _ctx = ExitStack()
    with tile.TileContext(nc) as tc:
        res = _ctx.enter_context(tc.tile_pool(name="res", bufs=1))
        xin = _ctx.enter_context(tc.tile_pool(name="xin", bufs=2))
        hwork = _ctx.enter_context(tc.tile_pool(name="hwork", bufs=1))
        scr = _ctx.enter_context(tc.tile_pool(name="scr", bufs=2))
        scrs = _ctx.enter_context(tc.tile_pool(name="scrs", bufs=2))
        pmm = _ctx.enter_context(tc.tile_pool(name="pmm", bufs=2, space="PSUM"))
        psm = _ctx.enter_context(tc.tile_pool(name="psm", bufs=3, space="PSUM"))
        pk1 = _ctx.enter_context(tc.tile_pool(name="pk1", bufs=1, space="PSUM"))

        def rtile(shape, dt, tag):
            return res.tile(shape, dt, tag=tag, name=tag)

        # ---- load constants / weights ----
        def load(dram, shape, dt, tag):
            t = rtile(shape, dt, tag)
            nc.sync.dma_start(out=t[:], in_=dram[:])
            return t

        w1c_h = load(w1c_h_d, [128, 4 * H1], F16, "w1ch")
        w1c_l = load(w1c_l_d, [128, 4 * H1], F16, "w1cl")
        w1a_h = load(w1a_h_d, [ACTD, H1], F16, "w1ah")
        w1a_l = load(w1a_l_d, [ACTD, H1], F16, "w1al")
        b1 = load(b1_d, [128, 4], F32, "b1")
        nb1 = load(nb1_d, [128, 4], F32, "nb1")
        w2_h = load(w2_h_d, [128, 8 * 128], F16, "w2h")
        w2_l = load(w2_l_d, [128, 8 * 128], F16, "w2l")
        b2 = load(b2_d, [128, 2], F32, "b2")
        nb2 = load(nb2_d, [128, 2], F32, "nb2")
        w3_h = load(w3_h_d, [128, 2 * EMB], F16, "w3h")
        w3_l = load(w3_l_d, [128, 2 * EMB], F16, "w3l")
        b3_h = load(b3_h_d, [1, EMB], F16, "b3h")
        b3_l = load(b3_l_d, [1, EMB], F16, "b3l")
        ent_h = load(ent_h_d, [EMB, K], F16, "enth")
        ent_l = load(ent_l_d, [EMB, K], F16, "entl")
        ensl_h = load(ensl_h_d, [EMB, 64], F16, "enslh")
        ensl_l = load(ensl_l_d, [EMB, 64], F16, "ensll")
        e_h = load(e_h_d, [128, 4 * EMB], F16, "eh")
        e_l = load(e_l_d, [128, 4 * EMB], F16, "el")
        wd1c_h = load(wd1c_h_d, [128, 8 * 128], F16, "wd1ch")
        wd1c_l = load(wd1c_l_d, [128, 8 * 128], F16, "wd1cl")
        wd1q = load(wd1q_d, [EMB, H2], F16, "wd1q")
        bd1 = load(bd1_d, [128, 2], F32, "bd1")
        nbd1 = load(nbd1_d, [128, 2], F32, "nbd1")
        wd2_h = load(wd2_h_d, [128, 8 * 128], F16, "wd2h")
        wd2_l = load(wd2_l_d, [128, 8 * 128], F16, "wd2l")
        bd2 = load(bd2_d, [128, 4], F32, "bd2")
        nbd2 = load(nbd2_d, [128, 4], F32, "nbd2")
        wd3_h = load(wd3_h_d, [128, 4 * ACTD], F16, "wd3h")
        wd3_l = load(wd3_l_d, [128, 4 * ACTD], F16, "wd3l")
        bd3_h = load(bd3_h_d, [1, ACTD], F16, "bd3h")
        bd3_l = load(bd3_l_d, [1, ACTD], F16, "bd3l")
        iota = load(iota_d, [128, K], F32, "iota")
        ident = load(ident_d, [128, 128], F16, "ident")

        ones_r = rtile([1, 128], F16, "ones_r")
        nc.vector.memset(ones_r[:], 1.0)
        ones_c = rtile([128, 1], F16, "ones_c")
        nc.vector.memset(ones_c[:], 1.0)
        ones_c32 = rtile([64, 1], F32, "ones_c32")
        nc.vector.memset(ones_c32[:], 1.0)
        eps30 = rtile([128, 1], F32, "eps30")
        nc.vector.memset(eps30[:], 1e-30)
        eps10 = rtile([128, 1], F32, "eps10")
        nc.vector.memset(eps10[:], 1e-10)

        # persistent state
        zsb = rtile([128, NT * EMB], F32, "zsb")
        q_sb = rtile([128, NT * EMB], F32, "qsb")
        q16_sb = rtile([128, NT * EMB], F16, "q16sb")
        idx_sb = rtile([128, NT], F32, "idxsb")
        recon_sb = rtile([128, NT * ACTD], F32, "reconsb")
        znh_sb = rtile([128, NT * EMB], F16, "znhsb")
        znl_sb = rtile([128, NT * EMB], F16, "znlsb")
        Dt = rtile([128, LH], F16, "Dt")

        # ---------- ELU' helper: out = elu(pre+b)+1, split hi/lo f16 ----------
        def elu_split(ps, fwid, bap, nbap, hi_out, lo_out):
            e = scr.tile([128, fwid], F32, tag="elu_e", name="elu_e", bufs=3)
            nc.scalar.activation(e[:], ps, Act.Exp, bias=bap, scale=1.0)
            em = scr.tile([128, fwid], F32, tag="elu_m", name="elu_m", bufs=3)
            nc.gpsimd.tensor_scalar(out=em[:], in0=e[:], scalar1=1.0,
                                    scalar2=bap, op0=Alu.min, op1=Alu.add)
            hf = scr.tile([128, fwid], F32, tag="elu_f", name="elu_f", bufs=3)
            nc.vector.scalar_tensor_tensor(out=hf[:], in0=ps, scalar=nbap,
                                           in1=em[:], op0=Alu.max, op1=Alu.add)
            nc.gpsimd.tensor_copy(hi_out, hf[:])
            nc.vector.tensor_tensor(out=lo_out, in0=hf[:], in1=hi_out,
                                    op=Alu.subtract)

        _PHASES.append(("enc", nc.next_id()))
        # ---------- encoder over blocks ----------
        for b in range(NBT):
            r0 = b * nblk
            xh = xin.tile([128, 4 * nblk], F16, tag="xh", name="xh", bufs=1)
            xl = xin.tile([128, 4 * nblk], F16, tag="xl", name="xl", bufs=1)
            for kc in range(4):
                nc.sync.dma_start_transpose(
                    xh[:, kc * nblk:(kc + 1) * nblk],
                    ch_d[r0:r0 + nblk, kc * 128:(kc + 1) * 128])
                nc.sync.dma_start_transpose(
                    xl[:, kc * nblk:(kc + 1) * nblk],
                    cl_d[r0:r0 + nblk, kc * 128:(kc + 1) * 128])
            aT_h = xin.tile([ACTD, nblk], F16, tag="aTh", name="aTh", bufs=2)
            aT_l = xin.tile([ACTD, nblk], F16, tag="aTl", name="aTl", bufs=2)
            nc.sync.dma_start_transpose(aT_h[:], ah_d[r0:r0 + nblk, :])
            nc.sync.dma_start_transpose(aT_l[:], al_d[r0:r0 + nblk, :])
            h1h = hwork.tile([128, 4 * nblk], F16, tag="hb_h", name="h1h")
            h1l = hwork.tile([128, 4 * nblk], F16, tag="hb_l", name="h1l")
            for ft in range(4):
                for hh in range(nblk // PF):
                    ps = pmm.tile([128, PF], F32, tag="pmm", name="pmm")
                    for sc_ in range(PF // 512):
                        o = hh * PF + sc_ * 512
                        po = ps[:, sc_ * 512:(sc_ + 1) * 512]
                        first = True
                        for kc in range(4):
                            wsl_h = w1c_h[:, kc * H1 + ft * 128:kc * H1 + ft * 128 + 128]
                            wsl_l = w1c_l[:, kc * H1 + ft * 128:kc * H1 + ft * 128 + 128]
                            rh = xh[:, kc * nblk + o:kc * nblk + o + 512]
                            rl = xl[:, kc * nblk + o:kc * nblk + o + 512]
                            nc.tensor.matmul(po, wsl_h, rh, start=first, stop=False)
                            first = False
                            nc.tensor.matmul(po, wsl_h, rl, start=False, stop=False)
                            nc.tensor.matmul(po, wsl_l, rh, start=False, stop=False)
                        ra_h = aT_h[:, o:o + 512]
                        ra_l = aT_l[:, o:o + 512]
                        wa_h = w1a_h[:, ft * 128:ft * 128 + 128]
                        wa_l = w1a_l[:, ft * 128:ft * 128 + 128]
                        nc.tensor.matmul(po, wa_h, ra_h, start=False, stop=False)
                        nc.tensor.matmul(po, wa_h, ra_l, start=False, stop=False)
                        nc.tensor.matmul(po, wa_l, ra_h, start=False, stop=True)
                    o = hh * PF
                    elu_split(ps[:], PF, b1[:, ft:ft + 1], nb1[:, ft:ft + 1],
                              h1h[:, ft * nblk + o:ft * nblk + o + PF],
                              h1l[:, ft * nblk + o:ft * nblk + o + PF])
            h2h = hwork.tile([128, 2 * nblk], F16, tag="hs_h", name="h2h")
            h2l = hwork.tile([128, 2 * nblk], F16, tag="hs_l", name="h2l")
            for ft in range(2):
                for hh in range(nblk // PF):
                    ps = pmm.tile([128, PF], F32, tag="pmm", name="pmm")
                    for sc_ in range(PF // 512):
                        o = hh * PF + sc_ * 512
                        po = ps[:, sc_ * 512:(sc_ + 1) * 512]
                        first = True
                        for kc in range(4):
                            wsl_h = w2_h[:, (kc * 2 + ft) * 128:(kc * 2 + ft) * 128 + 128]
                            wsl_l = w2_l[:, (kc * 2 + ft) * 128:(kc * 2 + ft) * 128 + 128]
                            rh = h1h[:, kc * nblk + o:kc * nblk + o + 512]
                            rl = h1l[:, kc * nblk + o:kc * nblk + o + 512]
                            nc.tensor.matmul(po, wsl_h, rh, start=first, stop=False)
                            first = False
                            nc.tensor.matmul(po, wsl_h, rl, start=False, stop=False)
                            nc.tensor.matmul(po, wsl_l, rh, start=False,
                                             stop=(kc == 3))
                    o = hh * PF
                    elu_split(ps[:], PF, b2[:, ft:ft + 1], nb2[:, ft:ft + 1],
                              h2h[:, ft * nblk + o:ft * nblk + o + PF],
                              h2l[:, ft * nblk + o:ft * nblk + o + PF])
            # L3 row-major: z tiles
            for tt in range(TPB):
                t = b * TPB + tt
                pz = psm.tile([128, 512], F32, tag="psm", name="psm")
                pzv = pz[:, 0:EMB]
                first = True
                for kc in range(2):
                    lh_ = h2h[:, kc * nblk + tt * 128:kc * nblk + tt * 128 + 128]
                    ll_ = h2l[:, kc * nblk + tt * 128:kc * nblk + tt * 128 + 128]
                    wh_ = w3_h[:, kc * EMB:(kc + 1) * EMB]
                    wl_ = w3_l[:, kc * EMB:(kc + 1) * EMB]
                    nc.tensor.matmul(pzv, lh_, wh_, start=first, stop=False)
                    first = False
                    nc.tensor.matmul(pzv, lh_, wl_, start=False, stop=False)
                    nc.tensor.matmul(pzv, ll_, wh_, start=False, stop=False)
                nc.tensor.matmul(pzv, ones_r[:], b3_h[:], start=False, stop=False)
                nc.tensor.matmul(pzv, ones_r[:], b3_l[:], start=False, stop=True)
                nc.vector.tensor_copy(zsb[:, t * EMB:(t + 1) * EMB], pzv)
            # normalize block -> zn hi/lo, store to DRAM
            t0 = b * TPB
            zblk = zsb[:, t0 * EMB:(t0 + TPB) * EMB]
            zsq = scr.tile([128, TPB * EMB], F32, tag="zsq", name="zsq", bufs=1)
            nc.scalar.square(zsq[:], zblk)
            nrm = scrs.tile([128, TPB], F32, tag="nrm", name="nrm")
            nc.vector.tensor_reduce(
                nrm[:], zsq[:].rearrange("p (t e) -> p t e", e=EMB),
                axis=mybir.AxisListType.X, op=Alu.add)
            srt = scrs.tile([128, TPB], F32, tag="srt", name="srt")
            nc.scalar.activation(srt[:], nrm[:], Act.Sqrt, bias=eps30[:], scale=1.0)
            s_ = scrs.tile([128, TPB], F32, tag="sinv", name="sinv")
            nc.vector.reciprocal(s_[:], srt[:])
            for tt in range(TPB):
                t = t0 + tt
                sl = slice(t * EMB, (t + 1) * EMB)
                nc.vector.tensor_scalar(
                    out=znh_sb[:, sl], in0=zsb[:, sl],
                    scalar1=s_[:, tt:tt + 1], scalar2=None, op0=Alu.mult)
                zf = scrs.tile([128, EMB], F32, tag="znf", name="znf")
                nc.vector.tensor_scalar(
                    out=zf[:], in0=zsb[:, sl],
                    scalar1=s_[:, tt:tt + 1], scalar2=None, op0=Alu.mult)
                nc.vector.tensor_tensor(out=znl_sb[:, sl], in0=zf[:],
                                        in1=znh_sb[:, sl], op=Alu.subtract)
            sth = scrs.tile([EMB, nblk], F16, tag="sth", name="sth", bufs=1)
            stl = scrs.tile([EMB, nblk], F16, tag="stl", name="stl", bufs=1)
            for tt in range(TPB):
                t = t0 + tt
                ptr = psm.tile([128, 1024], F16, tag="psmt", name="psmt", bufs=2)
                nc.tensor.transpose(ptr[0:EMB, 0:128],
                                    znh_sb[:, t * EMB:(t + 1) * EMB], ident[:])
                nc.vector.tensor_copy(sth[:, tt * 128:(tt + 1) * 128],
                                      ptr[0:EMB, 0:128])
                ptr2 = psm.tile([128, 1024], F16, tag="psmt", name="psmt", bufs=2)
                nc.tensor.transpose(ptr2[0:EMB, 0:128],
                                    znl_sb[:, t * EMB:(t + 1) * EMB], ident[:])
                nc.vector.tensor_copy(stl[:, tt * 128:(tt + 1) * 128],
                                      ptr2[0:EMB, 0:128])
            gp_ = r0 // PW
            go_ = r0 % PW
            nc.sync.dma_start(out=znh_p[gp_][:, go_:go_ + nblk], in_=sth[:])
            nc.sync.dma_start(out=znl_p[gp_][:, go_:go_ + nblk], in_=stl[:])

        _PHASES.append(("gather", nc.next_id()))
        # ---------- allgather zn ----------
        for gp_ in range(NGP):
            if _tlsim:
                for cc_ in range(NCORE):
                    nc.sync.dma_start(
                        out=zgh_p[gp_][cc_ * EMB:(cc_ + 1) * EMB, :],
                        in_=znh_p[gp_][:])
            else:
                nc.gpsimd.collective_compute(
                    "AllGather", Alu.bypass, replica_groups=groups,
                    ins=[znh_p[gp_].opt()], outs=[zgh_p[gp_].opt()])

        _PHASES.append(("Dt", nc.next_id()))
        # ---------- Dt: column-sharded distances [128, LH] ----------
        for zc in range(NFULL // ZCH):
            gr0 = zc * ZCH
            core_ix = gr0 // ns
            coff = gr0 % ns
            gp_ = coff // PW
            go_ = coff % PW
            zgT_h = xin.tile([EMB, ZCH], F16, tag="zgTh", name="zgTh")
            nc.sync.dma_start(
                out=zgT_h[:],
                in_=zgh_p[gp_][core_ix * EMB:(core_ix + 1) * EMB, go_:go_ + ZCH])
            half = (zc * ZCH) // (LH)
            base = (zc * ZCH) % LH
            for jj in range(ZCH // 512):
                pd = psm.tile([128, 512], F32, tag="psm", name="psm")
                rh = zgT_h[:, jj * 512:(jj + 1) * 512]
                po = pd[half * 64:(half + 1) * 64, :]
                tp = (0, 64) if half else None
                nc.tensor.matmul(po, ensl_h[:], rh, start=True, stop=True,
                                 tile_position=tp)
                o = base + jj * 512
                nc.scalar.copy(Dt[half * 64:(half + 1) * 64, o:o + 512],
                               pd[half * 64:(half + 1) * 64, :])

        _PHASES.append(("argmax", nc.next_id()))
        # ---------- fused per-block: argmax tiles then onehot@E (q) ----------
        pk = pk1.tile([1, K], F32, tag="counts", name="counts")
        for b in range(NBT):
            r0 = b * nblk
            for tt in range(TPB):
                t = b * TPB + tt
                if t % (ZCH // 128) == 0:
                    znT_h = xin.tile([EMB, ZCH], F16, tag="zgTh", name="zgTh")
                    znT_l = xin.tile([EMB, ZCH], F16, tag="zgTl", name="zgTl")
                    r0_ = t * 128
                    gp_ = r0_ // PW
                    go_ = r0_ % PW
                    nc.sync.dma_start(out=znT_h[:],
                                      in_=znh_p[gp_][:, go_:go_ + ZCH])
                    nc.sync.dma_start(out=znT_l[:],
                                      in_=znl_p[gp_][:, go_:go_ + ZCH])
                to = (t % (ZCH // 128)) * 128
                pD = psm.tile([128, 512], F32, tag="psm", name="psm")
                lh_ = znT_h[:, to:to + 128]
                ll_ = znT_l[:, to:to + 128]
                nc.tensor.matmul(pD[:], lh_, ent_h[:], start=True, stop=False)
                nc.tensor.matmul(pD[:], lh_, ent_l[:], start=False, stop=False)
                nc.tensor.matmul(pD[:], ll_, ent_h[:], start=False, stop=True)
                rmax = scrs.tile([128, 1], F32, tag="rmax", name="rmax", bufs=3)
                nc.vector.tensor_reduce(rmax[:], pD[:],
                                        axis=mybir.AxisListType.X, op=Alu.max)
                oh_tt = scr.tile([128, K], F16, tag="oht", name="oht", bufs=3)
                oh_t = oh_tt[:]
                nc.vector.tensor_scalar(out=oh_t, in0=pD[:], scalar1=rmax[:],
                                        scalar2=None, op0=Alu.is_ge)
                nc.tensor.matmul(pk[:], ones_c[:], oh_t, start=(t == 0),
                                 stop=(t == NT - 1))
                sidx = scr.tile([128, K], F32, tag="sidx", name="sidx", bufs=2)
                nc.vector.scalar_tensor_tensor(
                    out=sidx[:], in0=oh_t, scalar=1.0, in1=iota[:],
                    op0=Alu.mult, op1=Alu.mult,
                    accum_out=idx_sb[:, t:t + 1])
                nc.sync.dma_start(out=oh_d[t * 128:(t + 1) * 128, :], in_=oh_t)
            ohT = xin.tile([128, 4 * nblk], F16, tag="ohT", name="ohT")
            for kc in range(4):
                nc.sync.dma_start_transpose(
                    ohT[:, kc * nblk:(kc + 1) * nblk],
                    oh_d[r0:r0 + nblk, kc * 128:(kc + 1) * 128])
            for tt in range(TPB):
                t = b * TPB + tt
                pq = psm.tile([128, 512], F32, tag="psm", name="psm")
                pqv = pq[:, 0:EMB]
                for kc in range(4):
                    osl = ohT[:, kc * nblk + tt * 128:kc * nblk + tt * 128 + 128]
                    nc.tensor.matmul(pqv, osl, e_h[:, kc * EMB:(kc + 1) * EMB],
                                     start=(kc == 0), stop=False)
                    nc.tensor.matmul(pqv, osl, e_l[:, kc * EMB:(kc + 1) * EMB],
                                     start=False, stop=(kc == 3))
                nc.vector.tensor_copy(q_sb[:, t * EMB:(t + 1) * EMB], pqv)
                nc.vector.tensor_copy(q16_sb[:, t * EMB:(t + 1) * EMB], pqv)
                ptq = psm.tile([128, 1024], F16, tag="psmt", name="psmt", bufs=2)
                nc.tensor.transpose(ptq[0:EMB, 0:128],
                                    q16_sb[:, t * EMB:(t + 1) * EMB], ident[:])
                stq = scrs.tile([EMB, 128], F16, tag="stq", name="stq", bufs=2)
                nc.vector.tensor_copy(stq[:], ptq[0:EMB, 0:128])
                nc.sync.dma_start(out=q16_d[:, t * 128:(t + 1) * 128],
                                  in_=stq[:])
        idx_i = scrs.tile([128, NT], I32, tag="idxi", name="idxi", bufs=1)
        nc.vector.tensor_copy(idx_i[:], idx_sb[:])
        nc.sync.dma_start(out=idx_d.rearrange("(t p) -> p t", p=128),
                          in_=idx_i[:])
        nc.sync.dma_start(out=qst_d.rearrange("(t p) e -> p t e", p=128),
                          in_=q_sb[:].rearrange("p (t e) -> p t e", e=EMB))

        _PHASES.append(("contra", nc.next_id()))
        # ---------- contra stats on Dt ----------
        # segment top-8 candidates
        cand0 = rtile([128, NSEG * 8], F16, "cand0")
        for s in range(NSEG):
            nc.vector.max(cand0[:, s * 8:(s + 1) * 8],
                          Dt[:, s * SEG:(s + 1) * SEG])
        cand = rtile([64, CW], F16, "cand")
        nc.sync.dma_start(out=cand[:, 0:NSEG * 8], in_=cand0[0:64, :])
        nc.sync.dma_start(out=cand[:, NSEG * 8:CW], in_=cand0[64:128, :])

        # bisect for top-NPOS threshold on candidates
        tlo = rtile([64, 1], F32, "tlo")
        thi = rtile([64, 1], F32, "thi")
        nc.vector.memset(tlo[:], -1.0)
        nc.vector.memset(thi[:], 1.0)
        tmid = rtile([64, 1], F32, "tmid")
        for it in range(15):
            nc.vector.tensor_tensor(out=tmid[:], in0=tlo[:], in1=thi[:],
                                    op=Alu.add)
            nc.vector.tensor_scalar_mul(tmid[:], tmid[:], 0.5)
            scc = scrs.tile([64, CW], F16, tag="scc", name="scc", bufs=1)
            cntc = scrs.tile([64, 1], F32, tag="cntc", name="cntc")
            nc.vector.tensor_scalar(out=scc[:], in0=cand[:], scalar1=tmid[:],
                                    scalar2=None, op0=Alu.is_gt, op1=Alu.add,
                                    accum_out=cntc[:])
            sel = scrs.tile([64, 1], U8, tag="selc", name="selc")
            nc.vector.tensor_scalar(out=sel[:], in0=cntc[:],
                                    scalar1=float(NPOS), scalar2=None,
                                    op0=Alu.is_ge)
            nsel = scrs.tile([64, 1], U8, tag="nselc", name="nselc")
            nc.vector.tensor_scalar(out=nsel[:], in0=cntc[:],
                                    scalar1=float(NPOS), scalar2=None,
                                    op0=Alu.is_lt)
            nc.vector.select(tlo[:], sel[:], tmid[:], tlo[:])
            nc.vector.select(thi[:], nsel[:], tmid[:], thi[:])
        t128 = rtile([128, 1], F32, "t128")
        nc.vector.tensor_copy(t128[0:64, :], tlo[:])
        nc.sync.dma_start(out=t128[64:128, :], in_=tlo[:])

        # p = (sum relu(v - t) + NPOS*t)/NPOS   (8 chunks, ACT relu-accum)
        negt = rtile([128, 1], F32, "negt")
        nc.vector.tensor_scalar_mul(negt[:], t128[:], -1.0)
        pacc = rtile([128, NCHUNK], F32, "pacc")
        for cchunk in range(NCHUNK):
            so = scr.tile([128, CH], F16, tag="cscr", name="cscr")
            nc.scalar.activation(so[:], Dt[:, cchunk * CH:(cchunk + 1) * CH],
                                 Act.Relu, bias=negt[:], scale=1.0,
                                 accum_out=pacc[:, cchunk:cchunk + 1])
        psum_ = rtile([128, 1], F32, "psum_")
        nc.vector.tensor_reduce(psum_[:], pacc[:], axis=mybir.AxisListType.X,
                                op=Alu.add)
        psum2 = rtile([64, 1], F32, "psum2")
        nc.sync.dma_start(out=psum2[:], in_=psum_[64:128, :])
        nc.vector.tensor_tensor(out=psum2[:], in0=psum2[:], in1=psum_[0:64, :],
                                op=Alu.add)
        pval = rtile([128, 1], F32, "pval")
        nc.vector.tensor_scalar(out=pval[0:64, :], in0=psum2[:],
                                scalar1=1.0 / NPOS, scalar2=t128[0:64, :],
                                op0=Alu.mult, op1=Alu.add)

        # median bisect on strided subsample (merged count across halves)
        mlo = rtile([64, 1], F32, "mlo")
        mhi = rtile([64, 1], F32, "mhi")
        nc.vector.memset(mlo[:], -1.0)
        nc.vector.memset(mhi[:], 1.0)
        mmid = rtile([128, 1], F32, "mmid")
        sub_ap = Dt[:, 0:LH:SST]
        for it in range(16):
            nc.vector.tensor_tensor(out=mmid[0:64, :], in0=mlo[:], in1=mhi[:],
                                    op=Alu.add)
            nc.vector.tensor_scalar_mul(mmid[0:64, :], mmid[0:64, :], 0.5)
            nc.sync.dma_start(out=mmid[64:128, :], in_=mmid[0:64, :])
            scm = scr.tile([128, SUBW], F16, tag="cscr", name="cscr")
            cntm = scrs.tile([128, 1], F32, tag="cntm", name="cntm")
            nc.vector.tensor_scalar(out=scm[:], in0=sub_ap, scalar1=mmid[:],
                                    scalar2=None, op0=Alu.is_lt, op1=Alu.add,
                                    accum_out=cntm[:])
            cnt2 = scrs.tile([64, 1], F32, tag="cnt2", name="cnt2")
            nc.sync.dma_start(out=cnt2[:], in_=cntm[64:128, :])
            nc.vector.tensor_tensor(out=cnt2[:], in0=cnt2[:], in1=cntm[0:64, :],
                                    op=Alu.add)
            selm = scrs.tile([64, 1], U8, tag="selm", name="selm")
            nc.vector.tensor_scalar(out=selm[:], in0=cnt2[:],
                                    scalar1=float(2 * SUBW // 2), scalar2=None,
                                    op0=Alu.is_lt)
            nselm = scrs.tile([64, 1], U8, tag="nselm", name="nselm")
            nc.vector.tensor_scalar(out=nselm[:], in0=cnt2[:],
                                    scalar1=float(2 * SUBW // 2), scalar2=None,
                                    op0=Alu.is_ge)
            nc.vector.select(mlo[:], selm[:], mmid[0:64, :], mlo[:])
            nc.vector.select(mhi[:], nselm[:], mmid[0:64, :], mhi[:])
        mfin = rtile([128, 1], F32, "mfin")
        nc.vector.tensor_tensor(out=mfin[0:64, :], in0=mlo[:], in1=mhi[:],
                                op=Alu.add)
        nc.vector.tensor_scalar_mul(mfin[0:64, :], mfin[0:64, :], 0.5)
        nc.sync.dma_start(out=mfin[64:128, :], in_=mfin[0:64, :])

        # exp pass + masked sums: U = sum_{v<m} exp((v-m)/tau - 15), cnt_less
        bm = rtile([128, 1], F32, "bm")
        nc.vector.tensor_scalar_mul(bm[:], mfin[:], -1.0 / 0.07)
        uacc = rtile([128, NCHUNK], F32, "uacc")
        cacc = rtile([128, NCHUNK], F32, "cacc")
        for cchunk in range(NCHUNK):
            dsl = Dt[:, cchunk * CH:(cchunk + 1) * CH]
            ech = scr.tile([128, CH], F32, tag="csce", name="csce")
            nc.scalar.activation(ech[:], dsl, Act.Exp, bias=bm[:],
                                 scale=1.0 / 0.07)
            mch = scr.tile([128, CH], F16, tag="cscr", name="cscr")
            nc.vector.scalar_tensor_tensor(
                out=mch[:], in0=dsl, scalar=mfin[:], in1=ech[:],
                op0=Alu.is_lt, op1=Alu.mult,
                accum_out=uacc[:, cchunk:cchunk + 1])
            sch = scr.tile([128, CH], F16, tag="cscr", name="cscr")
            nc.vector.tensor_scalar(out=sch[:], in0=dsl, scalar1=mfin[:],
                                    scalar2=None, op0=Alu.is_lt, op1=Alu.add,
                                    accum_out=cacc[:, cchunk:cchunk + 1])
        u1 = rtile([128, 1], F32, "u1")
        c1 = rtile([128, 1], F32, "c1")
        nc.vector.tensor_reduce(u1[:], uacc[:], axis=mybir.AxisListType.X,
                                op=Alu.add)
        nc.vector.tensor_reduce(c1[:], cacc[:], axis=mybir.AxisListType.X,
                                op=Alu.add)
        u2 = rtile([64, 1], F32, "u2")
        c2 = rtile([64, 1], F32, "c2")
        nc.sync.dma_start(out=u2[:], in_=u1[64:128, :])
        nc.sync.dma_start(out=c2[:], in_=c1[64:128, :])
        nc.vector.tensor_tensor(out=u2[:], in0=u2[:], in1=u1[0:64, :], op=Alu.add)
        nc.vector.tensor_tensor(out=c2[:], in0=c2[:], in1=c1[0:64, :], op=Alu.add)
        # T = (U*e^15 + (N/2 - cnt)) * exp((m-p)/tau); ck = log1p(T)
        w2t = rtile([64, 1], F32, "w2t")
        nc.vector.tensor_scalar(out=w2t[:], in0=c2[:], scalar1=-1.0,
                                scalar2=float(NFULL // 2), op0=Alu.mult,
                                op1=Alu.add)
        t0_ = rtile([64, 1], F32, "t0_")
        nc.vector.tensor_tensor(out=t0_[:], in0=u2[:], in1=w2t[:], op=Alu.add)
        dmp = rtile([64, 1], F32, "dmp")
        nc.vector.tensor_tensor(out=dmp[:], in0=mfin[0:64, :], in1=pval[0:64, :],
                                op=Alu.subtract)
        g_ = rtile([64, 1], F32, "g_")
        nc.scalar.activation(g_[:], dmp[:], Act.Exp, bias=0.0, scale=1.0 / 0.07)
        tv = rtile([64, 1], F32, "tv")
        nc.vector.tensor_tensor(out=tv[:], in0=t0_[:], in1=g_[:], op=Alu.mult)
        ck = rtile([64, 1], F32, "ck")
        nc.scalar.activation(ck[:], tv[:], Act.Ln, bias=1.0, scale=1.0)
        pc = psm.tile([128, 512], F32, tag="psm", name="psm")
        nc.tensor.matmul(pc[0:1, 0:1], ck[:], ones_c32[:], start=True, stop=True)
        dbg = rtile([128, 16], F32, "dbg")
        nc.vector.memset(dbg[:], 0.0)
        nc.vector.tensor_copy(dbg[:, 0:1], t128[:])
        nc.vector.tensor_copy(dbg[0:64, 1:2], pval[0:64, :])
        nc.vector.tensor_copy(dbg[:, 2:3], mfin[:])
        nc.vector.tensor_copy(dbg[:, 3:4], u1[:])
        nc.vector.tensor_copy(dbg[:, 4:5], c1[:])
        nc.vector.tensor_copy(dbg[0:64, 5:6], u2[:])
        nc.vector.tensor_copy(dbg[0:64, 6:7], c2[:])
        nc.vector.tensor_copy(dbg[0:64, 7:8], tv[:])
        nc.vector.tensor_copy(dbg[0:64, 8:9], ck[:])
        nc.vector.tensor_copy(dbg[0:64, 9:10], g_[:])
        nc.vector.tensor_copy(dbg[0:64, 10:11], w2t[:])
        nc.vector.tensor_copy(dbg[0:64, 11:14], cand[:, 0:3])
        nc.vector.tensor_copy(dbg[:, 14:16], cand0[:, 0:2])
        nc.sync.dma_start(out=dbg_d[:], in_=dbg[:])

        _PHASES.append(("decoder", nc.next_id()))
        # ---------- decoder ----------
        for b in range(NBT):
            r0 = b * nblk
            xh = xin.tile([128, 4 * nblk], F16, tag="xh", name="xh", bufs=1)
            xl = xin.tile([128, 4 * nblk], F16, tag="xl", name="xl", bufs=1)
            for kc in range(4):
                nc.sync.dma_start_transpose(
                    xh[:, kc * nblk:(kc + 1) * nblk],
                    ch_d[r0:r0 + nblk, kc * 128:(kc + 1) * 128])
                nc.sync.dma_start_transpose(
                    xl[:, kc * nblk:(kc + 1) * nblk],
                    cl_d[r0:r0 + nblk, kc * 128:(kc + 1) * 128])
            qT = xin.tile([EMB, nblk], F16, tag="qT", name="qT", bufs=1)
            nc.sync.dma_start(out=qT[:], in_=q16_d[:, r0:r0 + nblk])
            hdh = hwork.tile([128, 2 * nblk], F16, tag="hs_h", name="hdh")
            hdl = hwork.tile([128, 2 * nblk], F16, tag="hs_l", name="hdl")
            for ft in range(2):
                for hh in range(nblk // PF):
                    ps = pmm.tile([128, PF], F32, tag="pmm", name="pmm")
                    for sc_ in range(PF // 512):
                        o = hh * PF + sc_ * 512
                        po = ps[:, sc_ * 512:(sc_ + 1) * 512]
                        first = True
                        for kc in range(4):
                            wh_ = wd1c_h[:, (kc * 2 + ft) * 128:(kc * 2 + ft) * 128 + 128]
                            rh = xh[:, kc * nblk + o:kc * nblk + o + 512]
                            rl = xl[:, kc * nblk + o:kc * nblk + o + 512]
                            nc.tensor.matmul(po, wh_, rh, start=first, stop=False)
                            first = False
                            nc.tensor.matmul(po, wh_, rl, start=False, stop=False)
                        nc.tensor.matmul(po, wd1q[:, ft * 128:ft * 128 + 128],
                                         qT[:, o:o + 512], start=False, stop=True)
                    o = hh * PF
                    elu_split(ps[:], PF, bd1[:, ft:ft + 1], nbd1[:, ft:ft + 1],
                              hdh[:, ft * nblk + o:ft * nblk + o + PF],
                              hdl[:, ft * nblk + o:ft * nblk + o + PF])
            h2dh = hwork.tile([128, 4 * nblk], F16, tag="hb_h", name="h2dh")
            h2dl = hwork.tile([128, 4 * nblk], F16, tag="hb_l", name="h2dl")
            for ft in range(4):
                for hh in range(nblk // PF):
                    ps = pmm.tile([128, PF], F32, tag="pmm", name="pmm")
                    for sc_ in range(PF // 512):
                        o = hh * PF + sc_ * 512
                        po = ps[:, sc_ * 512:(sc_ + 1) * 512]
                        first = True
                        for kc in range(2):
                            wh_ = wd2_h[:, (kc * 4 + ft) * 128:(kc * 4 + ft) * 128 + 128]
                            rh = hdh[:, kc * nblk + o:kc * nblk + o + 512]
                            rl = hdl[:, kc * nblk + o:kc * nblk + o + 512]
                            nc.tensor.matmul(po, wh_, rh, start=first, stop=False)
                            first = False
                            nc.tensor.matmul(po, wh_, rl, start=False,
                                             stop=(kc == 1))
                    o = hh * PF
                    elu_split(ps[:], PF, bd2[:, ft:ft + 1], nbd2[:, ft:ft + 1],
                              h2dh[:, ft * nblk + o:ft * nblk + o + PF],
                              h2dl[:, ft * nblk + o:ft * nblk + o + PF])
            for tt in range(TPB):
                t = b * TPB + tt
                pr = psm.tile([128, 512], F32, tag="psm", name="psm")
                prv = pr[:, 0:ACTD]
                first = True
                for kc in range(4):
                    lh_ = h2dh[:, kc * nblk + tt * 128:kc * nblk + tt * 128 + 128]
                    ll_ = h2dl[:, kc * nblk + tt * 128:kc * nblk + tt * 128 + 128]
                    wh_ = wd3_h[:, kc * ACTD:(kc + 1) * ACTD]
                    nc.tensor.matmul(prv, lh_, wh_, start=first, stop=False)
                    first = False
                    nc.tensor.matmul(prv, ll_, wh_, start=False, stop=False)
                nc.tensor.matmul(prv, ones_r[:], bd3_h[:], start=False, stop=False)
                nc.tensor.matmul(prv, ones_r[:], bd3_l[:], start=False, stop=True)
                nc.vector.tensor_copy(recon_sb[:, t * ACTD:(t + 1) * ACTD], prv)
        nc.sync.dma_start(out=recon_d.rearrange("(t p) e -> p t e", p=128),
                          in_=recon_sb[:].rearrange("p (t e) -> p t e", e=ACTD))

        _PHASES.append(("losses", nc.next_id()))
        # ---------- losses ----------
        dql = scr.tile([128, NT * EMB], F32, tag="lscr", name="lscr", bufs=1)
        nc.vector.tensor_tensor(out=dql[:], in0=q_sb[:], in1=zsb[:],
                                op=Alu.subtract)
        sq1 = rtile([128, 1], F32, "sq1")
        dqs = scr.tile([128, NT * EMB], F32, tag="csce", name="csce")
        nc.scalar.activation(dqs[:], dql[:], Act.Square, bias=0.0, scale=1.0,
                             accum_out=sq1[:])
        ps_s = psm.tile([128, 512], F32, tag="psm", name="psm")
        one128 = rtile([128, 1], F32, "one128")
        nc.vector.memset(one128[:], 1.0)
        nc.tensor.matmul(ps_s[0:1, 0:1], sq1[:], one128[:], start=True, stop=True)

        act_sb = scr.tile([128, NT * ACTD], F32, tag="lscr", name="lscr", bufs=1)
        nc.sync.dma_start(
            out=act_sb[:].rearrange("p (t e) -> p t e", e=ACTD),
            in_=a32_d.rearrange("(t p) e -> p t e", p=128))
        drl = scr.tile([128, NT * ACTD], F32, tag="csce", name="csce")
        nc.vector.tensor_tensor(out=drl[:], in0=recon_sb[:], in1=act_sb[:],
                                op=Alu.subtract)
        sr1 = rtile([128, 1], F32, "sr1")
        drs = scr.tile([128, NT * ACTD], F32, tag="lscr", name="lscr", bufs=1)
        nc.scalar.activation(drs[:], drl[:], Act.Square, bias=0.0, scale=1.0,
                             accum_out=sr1[:])
        ps_r = psm.tile([128, 512], F32, tag="psm", name="psm")
        nc.tensor.matmul(ps_r[0:1, 0:1], sr1[:], one128[:], start=True, stop=True)

        # ---------- pack scalars, allreduce, finalize ----------
        sci = rtile([1, 520], F32, "sci")
        nc.vector.memset(sci[:], 0.0)
        nc.vector.tensor_copy(sci[:, 0:1], ps_s[0:1, 0:1])
        nc.vector.tensor_copy(sci[:, 1:2], ps_r[0:1, 0:1])
        nc.vector.tensor_copy(sci[:, 2:3], pc[0:1, 0:1])
        nc.vector.tensor_copy(sci[:, 8:8 + K], pk[:])
        nc.sync.dma_start(out=sci_d[:], in_=sci[:])
        if _tlsim:
            nc.sync.dma_start(out=sco_d[:], in_=sci_d[:])
        else:
            nc.gpsimd.collective_compute(
                "AllReduce", Alu.add, replica_groups=groups,
                ins=[sci_d.opt()], outs=[sco_d.opt()])
        sco = rtile([1, 520], F32, "sco")
        nc.sync.dma_start(out=sco[:], in_=sco_d[:])

        scal = rtile([1, 8], F32, "scal")
        nc.vector.memset(scal[:], 0.0)
        # q_latent = S/(N*EMB); e_latent = CC*q_latent; recon = R/(N*ACT); contra/K
        nc.vector.tensor_scalar_mul(scal[:, 0:1], sco[:, 0:1],
                                    1.0 / (NFULL * EMB))
        nc.vector.tensor_scalar_mul(scal[:, 1:2], sco[:, 0:1],
                                    CC / (NFULL * EMB))
        nc.vector.tensor_scalar_mul(scal[:, 2:3], sco[:, 2:3], 1.0 / K)
        nc.vector.tensor_scalar_mul(scal[:, 4:5], sco[:, 1:2],
                                    1.0 / (NFULL * ACTD))
        # perplexity from counts (reshaped to [128, 4])
        cnt_t = scrs.tile([128, 4], F32, tag="cnt_t", name="cnt_t")
        nc.sync.dma_start(
            out=cnt_t[:],
            in_=sco_d[:, 8:8 + K].rearrange("o (p f) -> (o p) f", p=128))
        pr_ = scrs.tile([128, 4], F32, tag="pr_", name="pr_")
        nc.vector.tensor_scalar_mul(pr_[:], cnt_t[:], 1.0 / NFULL)
        lg_ = scrs.tile([128, 4], F32, tag="lg_", name="lg_")
        nc.scalar.activation(lg_[:], pr_[:], Act.Ln, bias=eps10[:], scale=1.0)
        pl_ = scrs.tile([128, 4], F32, tag="pl_", name="pl_")
        nc.vector.tensor_tensor(out=pl_[:], in0=pr_[:], in1=lg_[:], op=Alu.mult)
        ent_p = scrs.tile([128, 1], F32, tag="entp", name="entp")
        nc.vector.tensor_reduce(ent_p[:], pl_[:], axis=mybir.AxisListType.X,
                                op=Alu.add)
        ps_e = psm.tile([128, 512], F32, tag="psm", name="psm")
        nc.tensor.matmul(ps_e[0:1, 0:1], ent_p[:], one128[:], start=True,
                         stop=True)
        nc.scalar.activation(scal[:, 3:4], ps_e[0:1, 0:1], Act.Exp, bias=0.0,
                             scale=-1.0)
        nc.sync.dma_start(out=sc_d[:], in_=scal[:])

        _ctx.close()

    nc.compile()
    return nc


def build_null(ns, nblk):
    """Same I/O signature, minimal work — for transport-overhead deltas."""
    return build(ns, nblk, _null=True)


def host_prep(We1, be1, We2, be2, We3, be3, Wd1, bd1, Wd2, bd2, Wd3, bd3, E):
    """Host-side weight packing (f16 hi/lo splits, bias folds, transposes)."""
    o = {}
    w1a = We1[0:ACTD]                      # [12, 512]
    w1c = We1[ACTD:ACTD + COND]            # [512, 512]
    w1c_pack = np.concatenate([w1c[kc * 128:(kc + 1) * 128] for kc in range(4)],
                              axis=1)      # [128, 2048]
    o["w1c_h"], o["w1c_l"] = f16(w1c_pack), f16lo(w1c_pack)
    o["w1a_h"], o["w1a_l"] = f16(w1a), f16lo(w1a)
    o["b1"] = be1.reshape(4, 128).T.astype(np.float32).copy()
    o["nb1"] = -o["b1"]
    w2p = np.concatenate(
        [We2[kc * 128:(kc + 1) * 128, ft * 128:(ft + 1) * 128]
         for kc in range(4) for ft in range(2)], axis=1)
    o["w2_h"], o["w2_l"] = f16(w2p), f16lo(w2p)
    b2f = (be2 - We2.sum(0)).astype(np.float32)
    o["b2"] = b2f.reshape(2, 128).T.copy()
    o["nb2"] = -o["b2"]
    w3p = np.concatenate([We3[kc * 128:(kc + 1) * 128] for kc in range(2)],
                         axis=1)
    o["w3_h"], o["w3_l"] = f16(w3p), f16lo(w3p)
    b3f = (be3 - We3.sum(0)).astype(np.float32).reshape(1, EMB)
    o["b3_h"], o["b3_l"] = f16(b3f), f16lo(b3f)
    En = E / np.maximum(np.linalg.norm(E, axis=-1, keepdims=True), 1e-12)
    EnT = En.T.astype(np.float32)          # [16, 512]
    o["ent_h"], o["ent_l"] = f16(EnT), f16lo(EnT)
    ep = np.concatenate([E[kc * 128:(kc + 1) * 128] for kc in range(4)], axis=1)
    o["e_h"], o["e_l"] = f16(ep), f16lo(ep)
    wd1q = Wd1[0:EMB]                      # [16, 256]
    wd1c = Wd1[EMB:EMB + COND]             # [512, 256]
    wd1p = np.concatenate(
        [wd1c[kc * 128:(kc + 1) * 128, ft * 128:(ft + 1) * 128]
         for kc in range(4) for ft in range(2)], axis=1)
    o["wd1c_h"], o["wd1c_l"] = f16(wd1p), f16lo(wd1p)
    o["wd1q"] = f16(wd1q)
    o["bd1"] = bd1.reshape(2, 128).T.astype(np.float32).copy()
    o["nbd1"] = -o["bd1"]
    wd2p = np.concatenate(
        [Wd2[kc * 128:(kc + 1) * 128, ft * 128:(ft + 1) * 128]
         for kc in range(2) for ft in range(4)], axis=1)
    o["wd2_h"], o["wd2_l"] = f16(wd2p), f16lo(wd2p)
    bd2f = (bd2 - Wd2.sum(0)).astype(np.float32)
    o["bd2"] = bd2f.reshape(4, 128).T.copy()
    o["nbd2"] = -o["bd2"]
    wd3p = np.concatenate([Wd3[kc * 128:(kc + 1) * 128] for kc in range(4)],
                          axis=1)
    o["wd3_h"], o["wd3_l"] = f16(wd3p), f16lo(wd3p)
    bd3f = (bd3 - Wd3.sum(0)).astype(np.float32).reshape(1, ACTD)
    o["bd3_h"], o["bd3_l"] = f16(bd3f), f16lo(bd3f)
    o["iota"] = np.broadcast_to(np.arange(K, dtype=np.float32), (128, K)).copy()
    o["ident"] = np.eye(128, dtype=np.float16)
    o["_EnT"] = EnT
    return o


def make_in_maps(actions, conditions, wp, ns):
    maps = []
    EnT = wp["_EnT"]
    shared = {k: v for k, v in wp.items() if not k.startswith("_")}
    for c in range(NCORE):
        sl = slice(c * ns, (c + 1) * ns)
        a = np.asarray(actions[sl], np.float32)
        cd = np.asarray(conditions[sl], np.float32)
        m = dict(shared)
        m["a32"] = a
        m["ah"], m["al"] = f16(a), f16lo(a)
        m["ch"], m["cl"] = f16(cd), f16lo(cd)
        esl = EnT[:, c * 64:(c + 1) * 64]
        m["ensl_h"], m["ensl_l"] = f16(esl), f16lo(esl)
        maps.append(m)
    return maps


_NC_CACHE = {}


def _get_nc(ns, nblk):
    key = (ns, nblk)
    if key not in _NC_CACHE:
        _NC_CACHE[key] = build(ns, nblk)
    return _NC_CACHE[key]


def kernel(actions, conditions, We1, be1, We2, be2, We3, be3,
           Wd1, bd1, Wd2, bd2, Wd3, bd3, E, _trace=False):
    from concourse.bass_utils import run_bass_kernel_spmd
    ns = actions.shape[0] // NCORE
    nblk = min(512, ns)
    nc = _get_nc(ns, nblk)
    wp = host_prep(np.asarray(We1, np.float32), np.asarray(be1, np.float32),
                   np.asarray(We2, np.float32), np.asarray(be2, np.float32),
                   np.asarray(We3, np.float32), np.asarray(be3, np.float32),
                   np.asarray(Wd1, np.float32), np.asarray(bd1, np.float32),
                   np.asarray(Wd2, np.float32), np.asarray(bd2, np.float32),
                   np.asarray(Wd3, np.float32), np.asarray(bd3, np.float32),
                   np.asarray(E, np.float32))
    maps = make_in_maps(actions, conditions, wp, ns)
    res = run_bass_kernel_spmd(nc, maps, core_ids=list(range(NCORE)),
                               trace=_trace)
    r = res.results
    recon = np.concatenate([r[c]["recon"] for c in range(NCORE)], axis=0)
    q_st = np.concatenate([r[c]["qst"] for c in range(NCORE)], axis=0)
    idx = np.concatenate([r[c]["idx"] for c in range(NCORE)], axis=0)
    sc = r[0]["scal"]
    out = (recon.astype(np.float32), q_st.astype(np.float32),
           idx.astype(np.int32),
           np.float32(sc[0, 0]), np.float32(sc[0, 1]), np.float32(sc[0, 2]),
           np.float32(sc[0, 3]), np.float32(sc[0, 4]))
    if _trace:
        return out, res
    return out
